# revision 1
# baseline (speedup 1.0000x reference)
"""Causal GQA self-attention (B=2, S=2048, D=2048, H=16, KV=4) on 8 TRN2 cores.

Sharding: core = (b, g) with b = batch (2) x g = kv-head group (4).
Each core computes 4 q-heads / 1 kv-head for one batch and a partial
projection output [S, D]; host sums the 4 group partials per batch.

Per-core pipeline (all matmuls in float32r = full-rate, ~1e-4 precision):
  1. QKV: q/k/v in natural [s, d] layout (lhsT = xT blocks, rhs = W tiles).
     RMS-norm scale via ACT Square+accum; rope+norm fused into DVE
     scalar_tensor_tensor ops; PE-transpose q/k into [hd, S] layout.
  2. Attention per (q-slice t of 512, head h): scoresT[k,q] blocks via
     matmul(lhsT=kT block, rhs=qT slice); exp on ACT (no max subtraction:
     |score| <= gain*sqrt(hd)); causal via host-provided mask tiles;
     PV via matmul(lhsT=v block, rhs=probsT) -> outT[hd, q]; row sums via
     ones-vector matmul; normalize with reciprocal + partition_broadcast.
  3. Proj: out[s, dout] partial = sum_h yT[h].T @ WprojT[h] accumulated in
     PSUM, written to DRAM.
"""
import os
import sys

if '/opt/trn_rl_repo' not in sys.path:
    sys.path.insert(0, '/opt/trn_rl_repo')

import numpy as np

B, S, D = 2, 2048, 2048
NH_TOT, NKV_TOT, HD = 16, 4, 128
NH = 4                 # q heads per core
NT = S // 128          # 16 s-tiles
NC_ = D // 128         # 16 c-tiles
T = 4                  # q-slices of 512
SM = 1.0 / np.sqrt(HD)
EPS = float(np.finfo(np.float32).eps)
ROPE_BASE = 10000.0

_PROG = None


def _build_program():
    import concourse.bass as bass
    import concourse.mybir as mybir
    import concourse.tile as tile
    from concourse import bacc
    from concourse.alu_op_type import AluOpType

    F32 = mybir.dt.float32
    F32R = mybir.dt.float32r
    AF = mybir.ActivationFunctionType

    nc = bacc.Bacc("TRN2", target_bir_lowering=False, debug=False)

    XT = nc.dram_tensor("XT", [D, S], F32R, kind="ExternalInput")          # x[b].T
    WQ = nc.dram_tensor("WQ", [128, NC_, 512], F32R, kind="ExternalInput")  # Wq_g.T tiled [c_p, ci, dq]
    WKV = nc.dram_tensor("WKV", [128, NC_, 256], F32R, kind="ExternalInput")
    WP = nc.dram_tensor("WP", [128, NH, D], F32R, kind="ExternalInput")     # [c_in_head, h, dout]
    COS2 = nc.dram_tensor("COS2", [128, NT, HD], F32, kind="ExternalInput")
    SIN2 = nc.dram_tensor("SIN2", [128, NT, HD], F32, kind="ExternalInput")
    GSM = nc.dram_tensor("GSM", [1, NH], F32, kind="ExternalInput")         # gain*sm per head
    IDENT = nc.dram_tensor("IDENT", [128, 128], F32R, kind="ExternalInput")
    ONES = nc.dram_tensor("ONES", [128, 1], F32R, kind="ExternalInput")
    MASKS = nc.dram_tensor("MASKS", [4, 128, 512], F32R, kind="ExternalInput")
    Y = nc.dram_tensor("Y", [S, D], F32, kind="ExternalOutput")

    with tile.TileContext(nc) as tc:
        with (
            tc.tile_pool(name="const", bufs=1) as const,
            tc.tile_pool(name="w", bufs=4) as wpool,
            tc.tile_pool(name="stream", bufs=4) as stream,
            tc.tile_pool(name="small", bufs=3) as small,
            tc.tile_pool(name="norm", bufs=2) as normp,
            tc.tile_pool(name="rope", bufs=3) as ropep,
            tc.tile_pool(name="big", bufs=1) as big,
            tc.tile_pool(name="yt", bufs=2) as ytp,
            tc.tile_pool(name="probs", bufs=4) as probsp,
            tc.tile_pool(name="outsb", bufs=2) as outsb,
            tc.tile_pool(name="psA", bufs=2, space="PSUM") as psA,
            tc.tile_pool(name="psB", bufs=2, space="PSUM") as psB,
            tc.tile_pool(name="psS", bufs=2, space="PSUM") as psS,
        ):
            # small consts first (cheap), then weights in 4 chunks so the
            # first matmuls start after ~1.5MB of weight DMA, not 6MB
            ident = const.tile([128, 128], F32R)
            nc.sync.dma_start(ident[:], IDENT[:])
            ones = const.tile([128, 1], F32R)
            nc.sync.dma_start(ones[:], ONES[:])
            gsm = const.tile([1, NH], F32)
            nc.sync.dma_start(gsm[:], GSM[:])
            gsm_bc = const.tile([128, NH], F32)
            nc.gpsimd.partition_broadcast(gsm_bc[:], gsm[:])

            wqkv = []
            for c4 in range(4):
                wt = wpool.tile([128, 4, 768], F32R, tag="w")
                nc.scalar.dma_start(wt[:, :, 0:512], WQ[:, 4 * c4:4 * c4 + 4, :])
                nc.scalar.dma_start(wt[:, :, 512:768], WKV[:, 4 * c4:4 * c4 + 4, :])
                wqkv.append(wt)

            qT = big.tile([128, NH, S], F32R)
            kT = big.tile([128, S], F32R)
            v_nat = big.tile([128, NT, HD], F32R)

            cos2 = None
            sin2 = None
            masks = const.tile([128, 4, 512], F32R)

            # ---------------- phase 1: QKV + rms-norm + rope + transpose
            for si in range(NT):
                xs = stream.tile([128, NC_, 128], F32R, tag="xs")
                nc.sync.dma_start(
                    xs[:], XT[:, si * 128:(si + 1) * 128].rearrange("(a p) s -> p a s", p=128))
                q_ps = psA.tile([128, 512], F32, tag="A")
                kv_ps = psB.tile([128, 256], F32, tag="B")
                for ci in range(NC_):
                    nc.tensor.matmul(q_ps[:], xs[:, ci, :], wqkv[ci // 4][:, ci % 4, 0:512],
                                     start=(ci == 0), stop=(ci == NC_ - 1))
                    nc.tensor.matmul(kv_ps[:], xs[:, ci, :], wqkv[ci // 4][:, ci % 4, 512:768],
                                     start=(ci == 0), stop=(ci == NC_ - 1))
                if si == 0:
                    # cos/sin share the xs streaming slots; needed once rope
                    # starts, so posted behind the first x s-tile
                    cos2 = stream.tile([128, NT, HD], F32, tag="xs")
                    nc.sync.dma_start(cos2[:], COS2[:])
                    sin2 = stream.tile([128, NT, HD], F32, tag="xs")
                    nc.sync.dma_start(sin2[:], SIN2[:])
                    # masks are first needed by attention; scalar queue
                    nc.scalar.dma_start(masks[:], MASKS[:].transpose([1, 0, 2]))

                # sum of squares per head (q: 4 heads, k: 1)
                scr = small.tile([128, 128], F32, tag="scr")
                ssq = small.tile([128, 8], F32, tag="ssq")
                for h in range(NH):
                    nc.scalar.activation(scr[:], q_ps[:, h * 128:(h + 1) * 128],
                                         AF.Square, accum_out=ssq[:, h:h + 1])
                nc.scalar.activation(scr[:], kv_ps[:, 0:128], AF.Square,
                                     accum_out=ssq[:, 4:5])
                mn = small.tile([128, 8], F32, tag="mn")
                nc.vector.tensor_scalar(mn[:, 0:5], ssq[:, 0:5], 1.0 / HD, EPS,
                                        AluOpType.mult, AluOpType.add)
                rt = small.tile([128, 8], F32, tag="rt")
                nc.scalar.sqrt(rt[:, 0:5], mn[:, 0:5])
                rn = small.tile([128, 8], F32, tag="rn")
                nc.vector.reciprocal(rn[:, 0:5], rt[:, 0:5])
                qsc = small.tile([128, 4], F32, tag="qsc")
                nc.vector.tensor_tensor(qsc[:], rn[:, 0:4], gsm_bc[:], AluOpType.mult)

                # rope + norm-scale fused; then PE transpose into T layout
                for h in range(NH + 1):
                    if h < NH:
                        raw = q_ps[:, h * 128:(h + 1) * 128]
                        sc_ap = qsc[:, h:h + 1]
                    else:
                        raw = kv_ps[:, 0:128]
                        sc_ap = rn[:, 4:5]
                    tcs = ropep.tile([128, 128], F32, tag="tcs")
                    tsn = ropep.tile([128, 128], F32, tag="tsn")
                    nc.vector.scalar_tensor_tensor(
                        tcs[:], raw, sc_ap, cos2[:, si, :], AluOpType.mult, AluOpType.mult)
                    nc.vector.scalar_tensor_tensor(
                        tsn[:, 0:64], raw[:, 64:128], sc_ap, sin2[:, si, 0:64],
                        AluOpType.mult, AluOpType.mult)
                    nc.vector.scalar_tensor_tensor(
                        tsn[:, 64:128], raw[:, 0:64], sc_ap, sin2[:, si, 64:128],
                        AluOpType.mult, AluOpType.mult)
                    nat = ropep.tile([128, 128], F32R, tag="nat")
                    nc.gpsimd.tensor_tensor(nat[:], tcs[:], tsn[:], AluOpType.add)
                    tp = psS.tile([128, 128], F32R, tag="S")
                    nc.tensor.transpose(tp[:], nat[:], ident[:])
                    if h < NH:
                        nc.scalar.copy(qT[:, h, si * 128:(si + 1) * 128], tp[:])
                    else:
                        nc.scalar.copy(kT[:, si * 128:(si + 1) * 128], tp[:])

                # v: plain copy out of psum
                nc.scalar.copy(v_nat[:, si, :], kv_ps[:, 128:256])

            # proj weights into the freed w slots (4 dout-quarters)
            wp = []
            for dq in range(4):
                wt = wpool.tile([128, NH, 512], F32R, tag="w")
                nc.scalar.dma_start(wt[:], WP[:, :, dq * 512:(dq + 1) * 512])
                wp.append(wt)

            # ---------------- phase 2: attention, two heads interleaved
            pending_epilogue = [None]

            def flush_epilogue():
                if pending_epilogue[0] is not None:
                    pending_epilogue[0]()
                    pending_epilogue[0] = None

            for t in range(T):
                yt_t = ytp.tile([128, NH, 512], F32R, tag="yt")
                nblk = 4 * t + 4
                for hp in (0, 2):
                    o_ps = {}
                    rs_ps = {}
                    for h in (hp, hp + 1):
                        o_ps[h] = psA.tile([128, 512], F32, tag="A", name=f"o_ps_{t}_{h}")
                        rs_ps[h] = psB.tile([1, 512], F32, tag="B", name=f"rs_ps_{t}_{h}")
                    for j in range(nblk):
                        sc = psS.tile([128, 1024], F32, tag="S", name=f"sc_{t}_{hp}_{j}")
                        for u, h in enumerate((hp, hp + 1)):
                            nc.tensor.matmul(
                                sc[:, u * 512:(u + 1) * 512],
                                kT[:, j * 128:(j + 1) * 128],
                                qT[:, h, t * 512:(t + 1) * 512],
                                start=True, stop=True)
                        prb = probsp.tile([128, 1024], F32R, tag="probs", name=f"prb_{t}_{hp}_{j}")
                        off = j - 4 * t
                        # exp for both heads in one ACT call
                        nc.scalar.activation(prb[:], sc[:], AF.Exp)
                        if off >= 0:
                            w_ = (off + 1) * 128
                            for u in range(2):
                                nc.vector.tensor_tensor(
                                    prb[:, u * 512:u * 512 + w_],
                                    prb[:, u * 512:u * 512 + w_],
                                    masks[:, off, 0:w_], AluOpType.mult)
                        for u, h in enumerate((hp, hp + 1)):
                            nc.tensor.matmul(
                                o_ps[h][:], v_nat[:, j, :], prb[:, u * 512:(u + 1) * 512],
                                start=(j == 0), stop=(j == nblk - 1), skip_group_check=True)
                            nc.tensor.matmul(
                                rs_ps[h][:], ones[:], prb[:, u * 512:(u + 1) * 512],
                                start=(j == 0), stop=(j == nblk - 1), skip_group_check=True)
                        if j == 0:
                            flush_epilogue()

                    # evict psum (frees o/rs slots), normalize off the PE
                    # critical path; emission deferred into the next unit
                    def make_epilogue(o_ps=o_ps, rs_ps=rs_ps, yt_t=yt_t, hp=hp):
                        def ep():
                            for h in (hp, hp + 1):
                                nc.vector.tensor_copy(yt_t[:, h, :], o_ps[h][:])
                                rs_sb = normp.tile([1, 512], F32, tag="rssb")
                                nc.vector.tensor_copy(rs_sb[:], rs_ps[h][:])
                                rs_bc = normp.tile([128, 512], F32, tag="rsbc")
                                nc.gpsimd.partition_broadcast(rs_bc[:], rs_sb[:])
                                rcp_bc = normp.tile([128, 512], F32, tag="rcpbc")
                                nc.vector.reciprocal(rcp_bc[:], rs_bc[:])
                                nc.vector.tensor_tensor(
                                    yt_t[:, h, :], yt_t[:, h, :], rcp_bc[:], AluOpType.mult)
                        return ep
                    pending_epilogue[0] = make_epilogue()

                # ---------------- phase 3 (per t): projection for s-tiles 4t..4t+3
                flush_epilogue()
                for si in range(4 * t, 4 * t + 4):
                    sl = si - 4 * t
                    for dtp in range(2):
                        pj0 = psB.tile([128, 512], F32, tag="B")
                        pj1 = psB.tile([128, 512], F32, tag="B")
                        for h in range(NH):
                            lhs = yt_t[:, h, sl * 128:(sl + 1) * 128]
                            nc.tensor.matmul(pj0[:], lhs, wp[2 * dtp][:, h, :],
                                             start=(h == 0), stop=(h == NH - 1),
                                             skip_group_check=True)
                            nc.tensor.matmul(pj1[:], lhs, wp[2 * dtp + 1][:, h, :],
                                             start=(h == 0), stop=(h == NH - 1),
                                             skip_group_check=True)
                        for k_, pj in enumerate((pj0, pj1)):
                            ev = outsb.tile([128, 512], F32, tag="ev")
                            nc.vector.tensor_copy(ev[:], pj[:])
                            nc.sync.dma_start(
                                Y[si * 128:(si + 1) * 128,
                                  dtp * 1024 + k_ * 512:dtp * 1024 + (k_ + 1) * 512], ev[:])

    nc.compile()
    return nc


def _host_inputs(x, Wq, Wk, Wv, Wproj, q_gain):
    x = np.asarray(x, dtype=np.float32)
    Wq = np.asarray(Wq, dtype=np.float32)
    Wk = np.asarray(Wk, dtype=np.float32)
    Wv = np.asarray(Wv, dtype=np.float32)
    Wproj = np.asarray(Wproj, dtype=np.float32)
    q_gain = np.asarray(q_gain, dtype=np.float32)

    inv = (1.0 / ROPE_BASE ** (np.arange(0, HD, 2, dtype=np.float32) / HD)).astype(np.float32)
    ang = np.outer(np.arange(S, dtype=np.float32), inv)
    cos = np.cos(ang).astype(np.float32)
    sin = np.sin(ang).astype(np.float32)
    cos2 = np.concatenate([cos, cos], 1).reshape(NT, 128, HD).transpose(1, 0, 2).copy()
    sin2 = np.concatenate([sin, -sin], 1).reshape(NT, 128, HD).transpose(1, 0, 2).copy()

    qq = np.arange(512)[None, :]
    kk = np.arange(128)[:, None]
    masks = np.stack([(kk <= qq - off * 128).astype(np.float32) for off in range(4)])
    ident = np.eye(128, dtype=np.float32)
    ones = np.ones((128, 1), dtype=np.float32)

    in_maps = []
    for cid in range(8):
        b, g = cid // 4, cid % 4
        wq = Wq[g * 512:(g + 1) * 512, :].T            # [D, 512]
        wk = Wk[g * 128:(g + 1) * 128, :].T            # [D, 128]
        wv = Wv[g * 128:(g + 1) * 128, :].T
        wkv = np.concatenate([wk, wv], 1)              # [D, 256]
        wp = Wproj[:, g * 512:(g + 1) * 512].T         # [512, D] (c_local, dout)
        in_maps.append({
            "XT": np.ascontiguousarray(x[b].T),
            "WQ": np.ascontiguousarray(wq.reshape(NC_, 128, 512).transpose(1, 0, 2)),
            "WKV": np.ascontiguousarray(wkv.reshape(NC_, 128, 256).transpose(1, 0, 2)),
            "WP": np.ascontiguousarray(wp.reshape(NH, 128, D).transpose(1, 0, 2)),
            "COS2": cos2, "SIN2": sin2,
            "GSM": (q_gain[g * 4:(g + 1) * 4] * SM).reshape(1, NH).astype(np.float32),
            "IDENT": ident, "ONES": ones, "MASKS": masks,
        })
    return in_maps


def _get_prog():
    global _PROG
    if _PROG is None:
        _PROG = _build_program()
    return _PROG


def kernel(x, Wq, Wk, Wv, Wproj, q_gain, _trace=False, _tmpdir=None):
    from concourse.bass_utils import run_bass_kernel_spmd
    nc = _get_prog()
    in_maps = _host_inputs(x, Wq, Wk, Wv, Wproj, q_gain)
    kwargs = {}
    if _tmpdir is not None:
        os.makedirs(_tmpdir, exist_ok=True)
        kwargs["tmpdir"] = _tmpdir
    res = run_bass_kernel_spmd(nc, in_maps, list(range(8)), trace=_trace, **kwargs)
    y = np.empty((B, S, D), dtype=np.float32)
    for b in range(B):
        acc = res.results[4 * b]["Y"].astype(np.float32).copy()
        for g in range(1, 4):
            acc += res.results[4 * b + g]["Y"]
        y[b] = acc
    if _trace:
        kernel._last_result = res
    return y



# revision 8
# speedup vs baseline: 1.2875x; 1.2875x over previous
"""Causal GQA self-attention (B=2, S=2048, D=2048, H=16, KV=4) on 8 TRN2 cores.

Sharding: core = (b, g) with b = batch (2) x g = kv-head group (4).
Each core computes 4 q-heads / 1 kv-head for one batch and a partial
projection output [S, D] in bf16; host sums the 4 group partials per batch.

v2 changes vs baseline (496us):
  - all matmul operands bf16 (x, W, qT/kT/v, probs, yt, Wp); psum stays fp32.
  - epilogue: reciprocal on [1,512] BEFORE broadcast (was reciprocal of
    [128,512] = 56us DVE); chain is recip -> gpsimd bcast -> one DVE mult.
  - proj eviction via ACT copy (psum->bf16 sbuf) + dma; no DVE in proj path.
  - rope: batched ops across the 4 q heads via broadcast APs (3 DVE + 1
    gpsimd per s-tile instead of 12 DVE + 4 gpsimd).
  - causal diagonal trim: diag blocks only compute live q columns.
  - software-pipelined: transposes delayed one s-tile; attention exp/pv
    chain pipelined so PE never waits on ACT exp.
  - weight/x DMA split across 4 engine queues; first matmul after ~1.3MB.
"""
import os
import sys

if '/opt/trn_rl_repo' not in sys.path:
    sys.path.insert(0, '/opt/trn_rl_repo')

import numpy as np

B, S, D = 2, 2048, 2048
NH_TOT, NKV_TOT, HD = 16, 4, 128
NH = 4                 # q heads per core
NT = S // 128          # 16 s-tiles
NC_ = D // 128         # 16 c-tiles
T = 4                  # q-slices of 512
SM = 1.0 / np.sqrt(HD)
EPS = float(np.finfo(np.float32).eps)
ROPE_BASE = 10000.0

_PROG = None


def _build_program():
    import concourse.bass as bass
    import concourse.mybir as mybir
    import concourse.tile as tile
    from concourse import bacc
    from concourse.alu_op_type import AluOpType

    F32 = mybir.dt.float32
    BF16 = mybir.dt.bfloat16
    AF = mybir.ActivationFunctionType

    nc = bacc.Bacc("TRN2", target_bir_lowering=False, debug=False)

    XT = nc.dram_tensor("XT", [D, S], BF16, kind="ExternalInput")            # x[b].T
    WQ = nc.dram_tensor("WQ", [128, NC_, 512], BF16, kind="ExternalInput")   # Wq_g.T tiled [c_p, ci, dq]
    WKV = nc.dram_tensor("WKV", [128, NC_, 256], BF16, kind="ExternalInput")
    WP = nc.dram_tensor("WP", [4, 128, NH, 512], BF16, kind="ExternalInput")  # [dq, c_in_head, h, dout]
    COS2 = nc.dram_tensor("COS2", [128, NT, HD], F32, kind="ExternalInput")
    SIN2 = nc.dram_tensor("SIN2", [128, NT, HD], F32, kind="ExternalInput")
    GSM = nc.dram_tensor("GSM", [1, NH], F32, kind="ExternalInput")          # gain*sm per head
    IDENT = nc.dram_tensor("IDENT", [128, 128], BF16, kind="ExternalInput")
    ONES = nc.dram_tensor("ONES", [128, 1], BF16, kind="ExternalInput")
    TRI = nc.dram_tensor("TRI", [128, 128], BF16, kind="ExternalInput")      # lower-tri ones
    Y = nc.dram_tensor("Y", [S, D], BF16, kind="ExternalOutput")

    with tile.TileContext(nc) as tc:
        with (
            tc.tile_pool(name="const", bufs=1) as const,
            tc.tile_pool(name="w", bufs=4) as wpool,
            tc.tile_pool(name="stream", bufs=2) as stream,
            tc.tile_pool(name="small", bufs=3) as small,
            tc.tile_pool(name="norm", bufs=4) as normp,
            tc.tile_pool(name="rope", bufs=2) as ropep,
            tc.tile_pool(name="big", bufs=1) as big,
            tc.tile_pool(name="yt", bufs=2) as ytp,
            tc.tile_pool(name="probs", bufs=4) as probsp,
            tc.tile_pool(name="outsb", bufs=4) as outsb,
            tc.tile_pool(name="psA", bufs=2, space="PSUM") as psA,
            tc.tile_pool(name="psB", bufs=2, space="PSUM") as psB,
            tc.tile_pool(name="psS", bufs=2, space="PSUM") as psS,
        ):
            # --- startup DMA, spread over queues so the first matmul starts
            # after ~1.3MB: sync: x (in 512KB c-chunks); scalar: consts +
            # wqkv01 + cos/sin; vector: wqkv23; gpsimd: proj weights.
            def dma_xs(tile_, s0):
                for c4 in range(4):
                    nc.sync.dma_start(
                        tile_[:, 4 * c4:4 * c4 + 4, :],
                        XT[c4 * 512:(c4 + 1) * 512, s0:s0 + 512]
                        .rearrange("(a p) s -> p a s", p=128))

            xs4 = []
            xs0 = stream.tile([128, NC_, 512], BF16, tag="xs")
            dma_xs(xs0, 0)
            xs4.append(xs0)

            gsm = const.tile([1, NH], F32)
            nc.scalar.dma_start(gsm[:], GSM[:])
            ident = const.tile([128, 128], BF16)
            nc.scalar.dma_start(ident[:], IDENT[:])
            ones = const.tile([128, 1], BF16)
            nc.scalar.dma_start(ones[:], ONES[:])
            tri = const.tile([128, 128], BF16)
            nc.scalar.dma_start(tri[:], TRI[:])

            wqkv = []
            for c4 in range(4):
                wt = wpool.tile([128, 4, 768], BF16, tag="w")
                eng = (nc.scalar, nc.scalar, nc.gpsimd, nc.gpsimd)[c4]
                eng.dma_start(wt[:, :, 0:512], WQ[:, 4 * c4:4 * c4 + 4, :])
                eng.dma_start(wt[:, :, 512:768], WKV[:, 4 * c4:4 * c4 + 4, :])
                wqkv.append(wt)

            cos2 = const.tile([128, NT, HD], F32)
            nc.scalar.dma_start(cos2[:], COS2[:])
            sin2 = const.tile([128, NT, HD], F32)
            nc.scalar.dma_start(sin2[:], SIN2[:])

            gsm_bc = const.tile([128, NH], F32)
            nc.gpsimd.partition_broadcast(gsm_bc[:], gsm[:])

            # proj weights on the gpsimd queue (needed from ~40% in)
            wp = []
            for dq in range(4):
                wt = wpool.tile([128, NH, 512], BF16, tag="wp")
                nc.gpsimd.dma_start(wt[:], WP[dq])
                wp.append(wt)

            qT = big.tile([128, NH, S], BF16)
            kT = big.tile([128, S], BF16)
            v_nat = big.tile([128, NT, HD], BF16)

            # ---------------- phase 1: QKV + rms-norm + rope + transpose
            pending_tp = [None]

            def flush_tp():
                if pending_tp[0] is not None:
                    pending_tp[0]()
                    pending_tp[0] = None

            for si in range(NT):
                c4i = si // 4
                sl = si % 4
                if sl == 0 and c4i + 1 < 4:
                    nxt = stream.tile([128, NC_, 512], BF16, tag="xs")
                    dma_xs(nxt, (c4i + 1) * 512)
                    xs4.append(nxt)
                xs = xs4[c4i]
                q_ps = psA.tile([128, 512], F32, tag="A")
                kv_ps = psB.tile([128, 256], F32, tag="B")
                for ci in range(NC_):
                    nc.tensor.matmul(q_ps[:], xs[:, ci, sl * 128:(sl + 1) * 128],
                                     wqkv[ci // 4][:, ci % 4, 0:512],
                                     start=(ci == 0), stop=(ci == NC_ - 1))
                    nc.tensor.matmul(kv_ps[:], xs[:, ci, sl * 128:(sl + 1) * 128],
                                     wqkv[ci // 4][:, ci % 4, 512:768],
                                     start=(ci == 0), stop=(ci == NC_ - 1))

                # sum of squares per head (q: 4 heads, k: 1) on ACT
                scr = small.tile([128, 128], F32, tag="scr")
                ssq = small.tile([128, 8], F32, tag="ssq")
                for h in range(NH):
                    nc.scalar.activation(scr[:], q_ps[:, h * 128:(h + 1) * 128],
                                         AF.Square, accum_out=ssq[:, h:h + 1])
                nc.scalar.activation(scr[:], kv_ps[:, 0:128], AF.Square,
                                     accum_out=ssq[:, 4:5])
                mn = small.tile([128, 8], F32, tag="mn")
                nc.vector.tensor_scalar(mn[:, 0:5], ssq[:, 0:5], 1.0 / HD, EPS,
                                        AluOpType.mult, AluOpType.add)
                rt = small.tile([128, 8], F32, tag="rt")
                nc.scalar.sqrt(rt[:, 0:5], mn[:, 0:5])
                rn = small.tile([128, 8], F32, tag="rn")
                nc.vector.reciprocal(rn[:, 0:5], rt[:, 0:5])
                qsc = small.tile([128, 4], F32, tag="qsc")
                nc.vector.tensor_tensor(qsc[:], rn[:, 0:4], gsm_bc[:], AluOpType.mult)

                # batched rope for the 4 q heads:
                #   qs  = q_ps * qsc[head]      (scale, per-head broadcast AP)
                #   tcs = qs * cos[rep]         (full width)
                #   tsn = swap_halves(qs) * sin[rep]  (two half ops)
                #   nat = tcs + tsn  -> bf16    (gpsimd)
                qs = ropep.tile([128, 4, 128], F32, tag="qs")
                qsc_b = qsc[:, 0:4].unsqueeze(2).broadcast_to([128, 4, 128])
                q3 = q_ps[:].rearrange("p (h d) -> p h d", h=4)
                nc.vector.tensor_tensor(qs[:], q3, qsc_b, AluOpType.mult)
                cos_b = cos2[:, si, :].unsqueeze(1).broadcast_to([128, 4, 128])
                tcs = ropep.tile([128, 4, 128], F32, tag="tcs")
                nc.vector.tensor_tensor(tcs[:], qs[:], cos_b, AluOpType.mult)
                tsn = ropep.tile([128, 4, 128], F32, tag="tsn")
                sinA = sin2[:, si, 0:64].unsqueeze(1).broadcast_to([128, 4, 64])
                sinB = sin2[:, si, 64:128].unsqueeze(1).broadcast_to([128, 4, 64])
                nc.vector.tensor_tensor(tsn[:, :, 0:64], qs[:, :, 64:128], sinA,
                                        AluOpType.mult)
                nc.vector.tensor_tensor(tsn[:, :, 64:128], qs[:, :, 0:64], sinB,
                                        AluOpType.mult)
                natq = ropep.tile([128, 4, 128], BF16, tag="natq")
                nc.gpsimd.tensor_tensor(natq[:], tcs[:], tsn[:], AluOpType.add)

                # k rope (1 head): fused scale via scalar_tensor_tensor
                kcs = ropep.tile([128, 128], F32, tag="kcs")
                ksn = ropep.tile([128, 128], F32, tag="ksn")
                kraw = kv_ps[:, 0:128]
                nc.vector.scalar_tensor_tensor(
                    kcs[:], kraw, rn[:, 4:5], cos2[:, si, :], AluOpType.mult,
                    AluOpType.mult)
                nc.vector.scalar_tensor_tensor(
                    ksn[:, 0:64], kraw[:, 64:128], rn[:, 4:5], sin2[:, si, 0:64],
                    AluOpType.mult, AluOpType.mult)
                nc.vector.scalar_tensor_tensor(
                    ksn[:, 64:128], kraw[:, 0:64], rn[:, 4:5], sin2[:, si, 64:128],
                    AluOpType.mult, AluOpType.mult)
                natk = ropep.tile([128, 128], BF16, tag="natk")
                nc.gpsimd.tensor_tensor(natk[:], kcs[:], ksn[:], AluOpType.add)

                # v: ACT copy out of psum (fp32 -> bf16)
                nc.scalar.copy(v_nat[:, si, :], kv_ps[:, 128:256])

                # transposes delayed one s-tile so PE never waits on rope
                flush_tp()

                def make_tp(si=si, natq=natq, natk=natk):
                    def tp_():
                        for h in range(NH):
                            tp = psS.tile([128, 128], BF16, tag="S", name=f"tpq_{si}_{h}")
                            nc.tensor.transpose(tp[:], natq[:, h, :], ident[:])
                            nc.vector.tensor_copy(qT[:, h, si * 128:(si + 1) * 128], tp[:])
                        tp = psS.tile([128, 128], BF16, tag="S", name=f"tpk_{si}")
                        nc.tensor.transpose(tp[:], natk[:], ident[:])
                        nc.vector.tensor_copy(kT[:, si * 128:(si + 1) * 128], tp[:])
                    return tp_
                pending_tp[0] = make_tp()
            flush_tp()

            # ---------------- phase 2: attention, two heads interleaved,
            # software-pipelined: emit sc(j), exp(j), then pv/rs(j-1).
            pending_epilogue = [None]

            def flush_epilogue():
                if pending_epilogue[0] is not None:
                    pending_epilogue[0]()
                    pending_epilogue[0] = None

            for t in range(T):
                yt_t = ytp.tile([128, NH, 512], BF16, tag="yt")
                nblk = 4 * t + 4
                for hp in (0, 2):
                    o_ps = {}
                    rs_ps = {}
                    for h in (hp, hp + 1):
                        o_ps[h] = psA.tile([128, 512], F32, tag="A", name=f"o_ps_{t}_{h}")
                        rs_ps[h] = psB.tile([1, 512], F32, tag="B", name=f"rs_ps_{t}_{h}")

                    sc_tiles = {}
                    prb_tiles = {}

                    def emit_sc(j, t=t, hp=hp, sc_tiles=sc_tiles, prb_tiles=prb_tiles):
                        off = j - 4 * t
                        q0 = max(off, 0) * 128   # live q start within the slice
                        sc = psS.tile([128, 1024], F32, tag="S", name=f"sc_{t}_{hp}_{j}")
                        for u, h in enumerate((hp, hp + 1)):
                            nc.tensor.matmul(
                                sc[:, u * 512 + q0:(u + 1) * 512],
                                kT[:, j * 128:(j + 1) * 128],
                                qT[:, h, t * 512 + q0:(t + 1) * 512],
                                start=True, stop=True, skip_group_check=True)
                        prb = probsp.tile([128, 1024], BF16, tag="probs",
                                          name=f"prb_{t}_{hp}_{j}")
                        if off <= 0:
                            nc.scalar.activation(prb[:], sc[:], AF.Exp)
                        else:
                            for u in range(2):
                                nc.scalar.activation(
                                    prb[:, u * 512 + q0:(u + 1) * 512],
                                    sc[:, u * 512 + q0:(u + 1) * 512], AF.Exp)
                        if off >= 0:
                            # mask the 128-wide diagonal sub-block
                            for u in range(2):
                                nc.vector.tensor_tensor(
                                    prb[:, u * 512 + q0:u * 512 + q0 + 128],
                                    prb[:, u * 512 + q0:u * 512 + q0 + 128],
                                    tri[:], AluOpType.mult)
                        sc_tiles[j] = sc
                        prb_tiles[j] = prb

                    def emit_pvrs(j, t=t, hp=hp, o_ps=o_ps, rs_ps=rs_ps,
                                  prb_tiles=prb_tiles, nblk=nblk):
                        off = j - 4 * t
                        q0 = max(off, 0) * 128
                        prb = prb_tiles.pop(j)
                        first = (j == 0)
                        last = (j == nblk - 1)
                        for u, h in enumerate((hp, hp + 1)):
                            nc.tensor.matmul(
                                o_ps[h][:, q0:512], v_nat[:, j, :],
                                prb[:, u * 512 + q0:(u + 1) * 512],
                                start=first, stop=last, skip_group_check=True)
                            nc.tensor.matmul(
                                rs_ps[h][:, q0:512], ones[:],
                                prb[:, u * 512 + q0:(u + 1) * 512],
                                start=first, stop=last, skip_group_check=True)

                    emit_sc(0)
                    for j in range(1, nblk):
                        emit_sc(j)
                        emit_pvrs(j - 1)
                        if j == 1:
                            flush_epilogue()
                    emit_pvrs(nblk - 1)
                    if nblk == 1:
                        flush_epilogue()

                    # normalize: recip on [1,512], broadcast, single mult
                    def make_epilogue(o_ps=o_ps, rs_ps=rs_ps, yt_t=yt_t, hp=hp):
                        def ep():
                            for h in (hp, hp + 1):
                                rcp = normp.tile([1, 512], F32, tag="rcp")
                                nc.vector.reciprocal(rcp[:], rs_ps[h][:])
                                rcp_bc = normp.tile([128, 512], F32, tag="rcpbc")
                                nc.gpsimd.partition_broadcast(rcp_bc[:], rcp[:])
                                nc.vector.tensor_tensor(
                                    yt_t[:, h, :], o_ps[h][:], rcp_bc[:],
                                    AluOpType.mult)
                        return ep
                    pending_epilogue[0] = make_epilogue()

                # ---------------- phase 3 (per t): projection for s-tiles 4t..4t+3
                flush_epilogue()
                for si in range(4 * t, 4 * t + 4):
                    sl = si - 4 * t
                    for dtp in range(2):
                        pj0 = psB.tile([128, 512], F32, tag="B")
                        pj1 = psB.tile([128, 512], F32, tag="B")
                        for h in range(NH):
                            lhs = yt_t[:, h, sl * 128:(sl + 1) * 128]
                            nc.tensor.matmul(pj0[:], lhs, wp[2 * dtp][:, h, :],
                                             start=(h == 0), stop=(h == NH - 1),
                                             skip_group_check=True)
                            nc.tensor.matmul(pj1[:], lhs, wp[2 * dtp + 1][:, h, :],
                                             start=(h == 0), stop=(h == NH - 1),
                                             skip_group_check=True)
                        for k_, pj in enumerate((pj0, pj1)):
                            ev = outsb.tile([128, 512], BF16, tag="ev")
                            nc.scalar.copy(ev[:], pj[:])
                            nc.sync.dma_start(
                                Y[si * 128:(si + 1) * 128,
                                  dtp * 1024 + k_ * 512:dtp * 1024 + (k_ + 1) * 512], ev[:])

    nc.compile()
    return nc


def _host_inputs(x, Wq, Wk, Wv, Wproj, q_gain):
    import ml_dtypes
    bf16 = ml_dtypes.bfloat16

    x = np.asarray(x, dtype=np.float32)
    Wq = np.asarray(Wq, dtype=np.float32)
    Wk = np.asarray(Wk, dtype=np.float32)
    Wv = np.asarray(Wv, dtype=np.float32)
    Wproj = np.asarray(Wproj, dtype=np.float32)
    q_gain = np.asarray(q_gain, dtype=np.float32)

    inv = (1.0 / ROPE_BASE ** (np.arange(0, HD, 2, dtype=np.float32) / HD)).astype(np.float32)
    ang = np.outer(np.arange(S, dtype=np.float32), inv)
    cos = np.cos(ang).astype(np.float32)
    sin = np.sin(ang).astype(np.float32)
    cos2 = np.concatenate([cos, cos], 1).reshape(NT, 128, HD).transpose(1, 0, 2).copy()
    sin2 = np.concatenate([sin, -sin], 1).reshape(NT, 128, HD).transpose(1, 0, 2).copy()

    qq = np.arange(128)[None, :]
    kk = np.arange(128)[:, None]
    tri = (kk <= qq).astype(bf16)
    ident = np.eye(128, dtype=bf16)
    ones = np.ones((128, 1), dtype=bf16)

    xTb = [np.ascontiguousarray(x[b].T).astype(bf16) for b in range(B)]

    in_maps = []
    for cid in range(8):
        b, g = cid // 4, cid % 4
        wq = Wq[g * 512:(g + 1) * 512, :].T            # [D, 512]
        wk = Wk[g * 128:(g + 1) * 128, :].T            # [D, 128]
        wv = Wv[g * 128:(g + 1) * 128, :].T
        wkv = np.concatenate([wk, wv], 1)              # [D, 256]
        wp = Wproj[:, g * 512:(g + 1) * 512].T         # [512, D] (c_local, dout)
        in_maps.append({
            "XT": xTb[b],
            "WQ": np.ascontiguousarray(
                wq.reshape(NC_, 128, 512).transpose(1, 0, 2)).astype(bf16),
            "WKV": np.ascontiguousarray(
                wkv.reshape(NC_, 128, 256).transpose(1, 0, 2)).astype(bf16),
            "WP": np.ascontiguousarray(
                wp.reshape(NH, 128, 4, 512).transpose(2, 1, 0, 3)).astype(bf16),
            "COS2": cos2, "SIN2": sin2,
            "GSM": (q_gain[g * 4:(g + 1) * 4] * SM).reshape(1, NH).astype(np.float32),
            "IDENT": ident, "ONES": ones, "TRI": tri,
        })
    return in_maps


def _get_prog():
    global _PROG
    if _PROG is None:
        _PROG = _build_program()
    return _PROG


def kernel(x, Wq, Wk, Wv, Wproj, q_gain, _trace=False, _tmpdir=None):
    from concourse.bass_utils import run_bass_kernel_spmd
    nc = _get_prog()
    in_maps = _host_inputs(x, Wq, Wk, Wv, Wproj, q_gain)
    kwargs = {}
    if _tmpdir is not None:
        os.makedirs(_tmpdir, exist_ok=True)
        kwargs["tmpdir"] = _tmpdir
    res = run_bass_kernel_spmd(nc, in_maps, list(range(8)), trace=_trace, **kwargs)
    y = np.empty((B, S, D), dtype=np.float32)
    for b in range(B):
        acc = res.results[4 * b]["Y"].astype(np.float32)
        for g in range(1, 4):
            acc = acc + res.results[4 * b + g]["Y"].astype(np.float32)
        y[b] = acc
    if _trace:
        kernel._last_result = res
    return y


# revision 19
# speedup vs baseline: 1.6188x; 1.2574x over previous
"""Causal GQA self-attention (B=2, S=2048, D=2048, H=16, KV=4) on 8 TRN2 cores.

Sharding: core = (b, g) with b = batch (2) x g = kv-head group (4).
Each core computes 4 q-heads / 1 kv-head for one batch and a partial
projection output [S, D] in bf16; host sums the 4 group partials per batch.

v2 changes vs baseline (496us):
  - all matmul operands bf16 (x, W, qT/kT/v, probs, yt, Wp); psum stays fp32.
  - epilogue: reciprocal on [1,512] BEFORE broadcast (was reciprocal of
    [128,512] = 56us DVE); chain is recip -> gpsimd bcast -> one DVE mult.
  - proj eviction via ACT copy (psum->bf16 sbuf) + dma; no DVE in proj path.
  - rope: batched ops across the 4 q heads via broadcast APs (3 DVE + 1
    gpsimd per s-tile instead of 12 DVE + 4 gpsimd).
  - causal diagonal trim: diag blocks only compute live q columns.
  - software-pipelined: transposes delayed one s-tile; attention exp/pv
    chain pipelined so PE never waits on ACT exp.
  - weight/x DMA split across 4 engine queues; first matmul after ~1.3MB.
"""
import os
import sys

if '/opt/trn_rl_repo' not in sys.path:
    sys.path.insert(0, '/opt/trn_rl_repo')

import numpy as np

B, S, D = 2, 2048, 2048
NH_TOT, NKV_TOT, HD = 16, 4, 128
NH = 4                 # q heads per core
NT = S // 128          # 16 s-tiles
NC_ = D // 128         # 16 c-tiles
T = 4                  # q-slices of 512
SM = 1.0 / np.sqrt(HD)
EPS = float(np.finfo(np.float32).eps)
ROPE_BASE = 10000.0

_PROG = None


def _build_program():
    import concourse.bass as bass
    import concourse.mybir as mybir
    import concourse.tile as tile
    from concourse import bacc
    from concourse.alu_op_type import AluOpType

    F32 = mybir.dt.float32
    BF16 = mybir.dt.bfloat16
    AF = mybir.ActivationFunctionType

    nc = bacc.Bacc("TRN2", target_bir_lowering=False, debug=False)

    # x[b].T pre-tiled: [si4, c4, p, a, s] so each (si4,c4) load is 128x4KB
    XT = nc.dram_tensor("XT", [4, 4, 128, 4, 512], BF16, kind="ExternalInput")
    WQ = nc.dram_tensor("WQ", [128, NC_, 512], BF16, kind="ExternalInput")   # Wq_g.T tiled [c_p, ci, dq]
    WKV = nc.dram_tensor("WKV", [128, NC_, 256], BF16, kind="ExternalInput")
    WP = nc.dram_tensor("WP", [4, 128, NH, 512], BF16, kind="ExternalInput")  # [dq, c_in_head, h, dout]
    COS2 = nc.dram_tensor("COS2", [128, NT, HD], F32, kind="ExternalInput")
    SIN2 = nc.dram_tensor("SIN2", [128, NT, HD], F32, kind="ExternalInput")
    GSM = nc.dram_tensor("GSM", [1, NH], F32, kind="ExternalInput")          # gain*sm per head
    IDENT = nc.dram_tensor("IDENT", [128, 128], BF16, kind="ExternalInput")
    ONESQ = nc.dram_tensor("ONESQ", [128, 128], BF16, kind="ExternalInput")
    TRI = nc.dram_tensor("TRI", [128, 128], BF16, kind="ExternalInput")      # lower-tri ones
    Y = nc.dram_tensor("Y", [S, D], BF16, kind="ExternalOutput")

    with tile.TileContext(nc) as tc:
        with (
            tc.tile_pool(name="const", bufs=1) as const,
            tc.tile_pool(name="w", bufs=4) as wpool,
            tc.tile_pool(name="stream", bufs=2) as stream,
            tc.tile_pool(name="small", bufs=3) as small,
            tc.tile_pool(name="norm", bufs=4) as normp,
            tc.tile_pool(name="rope", bufs=2) as ropep,
            tc.tile_pool(name="big", bufs=1) as big,
            tc.tile_pool(name="yt", bufs=2) as ytp,
            tc.tile_pool(name="probs", bufs=4) as probsp,
            tc.tile_pool(name="outsb", bufs=4) as outsb,
            tc.tile_pool(name="psA", bufs=2, space="PSUM") as psA,
            tc.tile_pool(name="psB", bufs=2, space="PSUM") as psB,
            tc.tile_pool(name="psS", bufs=2, space="PSUM") as psS,
        ):
            # --- startup DMA, spread over queues so the first matmul starts
            # after ~800KB: sync: x chunks + wqkv3; scalar: consts + wqkv01
            # + cos/sin; gpsimd: wqkv2 + proj weights.
            xs4 = []
            xs0 = stream.tile([128, NC_, 512], BF16, tag="xs")
            for c4 in range(4):
                nc.sync.dma_start(xs0[:, 4 * c4:4 * c4 + 4, :], XT[0, c4])
            xs4.append(xs0)

            gsm = const.tile([1, NH], F32)
            nc.scalar.dma_start(gsm[:], GSM[:])
            ident = const.tile([128, 128], BF16)
            nc.scalar.dma_start(ident[:], IDENT[:])
            onesq = const.tile([128, 128], BF16)
            nc.scalar.dma_start(onesq[:], ONESQ[:])
            tri = const.tile([128, 128], BF16)
            nc.scalar.dma_start(tri[:], TRI[:])

            wqkv = []
            for c4 in range(4):
                wt = wpool.tile([128, 4, 768], BF16, tag="w")
                eng = (nc.scalar, nc.scalar, nc.gpsimd, nc.sync)[c4]
                eng.dma_start(wt[:, :, 0:512], WQ[:, 4 * c4:4 * c4 + 4, :])
                eng.dma_start(wt[:, :, 512:768], WKV[:, 4 * c4:4 * c4 + 4, :])
                wqkv.append(wt)

            cos2 = const.tile([128, NT, HD], F32)
            nc.scalar.dma_start(cos2[:], COS2[:])
            sin2 = const.tile([128, NT, HD], F32)
            nc.scalar.dma_start(sin2[:], SIN2[:])

            gsm_bc = const.tile([128, NH], F32)
            nc.gpsimd.partition_broadcast(gsm_bc[:], gsm[:])

            # proj weights on the gpsimd queue (needed from ~40% in)
            wp = []
            for dq in range(4):
                wt = wpool.tile([128, NH, 512], BF16, tag="wp")
                nc.gpsimd.dma_start(wt[:], WP[dq])
                wp.append(wt)

            qT = big.tile([128, NH, S], BF16)
            kT = big.tile([128, S], BF16)
            v_nat = big.tile([128, NT, HD], BF16)

            # ---------------- phase 1: QKV + rms-norm + rope + transpose
            pending_tp = [None]

            def flush_tp():
                if pending_tp[0] is not None:
                    pending_tp[0]()
                    pending_tp[0] = None

            for si in range(NT):
                c4i = si // 4
                sl = si % 4
                if sl == 0 and c4i + 1 < 4:
                    nxt = stream.tile([128, NC_, 512], BF16, tag="xs")
                    for c4 in range(4):
                        nc.sync.dma_start(nxt[:, 4 * c4:4 * c4 + 4, :],
                                          XT[c4i + 1, c4])
                    xs4.append(nxt)
                xs = xs4[c4i]
                q_ps = psA.tile([128, 512], F32, tag="A")
                kv_ps = psB.tile([128, 256], F32, tag="B")
                for ci in range(NC_):
                    nc.tensor.matmul(q_ps[:], xs[:, ci, sl * 128:(sl + 1) * 128],
                                     wqkv[ci // 4][:, ci % 4, 0:512],
                                     start=(ci == 0), stop=(ci == NC_ - 1))
                for ci in range(NC_):
                    nc.tensor.matmul(kv_ps[:], xs[:, ci, sl * 128:(sl + 1) * 128],
                                     wqkv[ci // 4][:, ci % 4, 512:768],
                                     start=(ci == 0), stop=(ci == NC_ - 1))

                # sum of squares per head (q: 4 heads, k: 1) on ACT
                scr = small.tile([128, 128], F32, tag="scr")
                ssq = small.tile([128, 8], F32, tag="ssq")
                for h in range(NH):
                    nc.scalar.activation(scr[:], q_ps[:, h * 128:(h + 1) * 128],
                                         AF.Square, accum_out=ssq[:, h:h + 1])
                nc.scalar.activation(scr[:], kv_ps[:, 0:128], AF.Square,
                                     accum_out=ssq[:, 4:5])
                mn = small.tile([128, 8], F32, tag="mn")
                nc.vector.tensor_scalar(mn[:, 0:5], ssq[:, 0:5], 1.0 / HD, EPS,
                                        AluOpType.mult, AluOpType.add)
                rt = small.tile([128, 8], F32, tag="rt")
                nc.scalar.sqrt(rt[:, 0:5], mn[:, 0:5])
                rn = small.tile([128, 8], F32, tag="rn")
                nc.vector.reciprocal(rn[:, 0:5], rt[:, 0:5])
                qsc = small.tile([128, 4], F32, tag="qsc")
                nc.vector.tensor_tensor(qsc[:], rn[:, 0:4], gsm_bc[:], AluOpType.mult)

                # batched rope for the 4 q heads:
                #   qs  = q_ps * qsc[head]      (scale, per-head broadcast AP)
                #   tcs = qs * cos[rep]         (full width)
                #   tsn = swap_halves(qs) * sin[rep]  (two half ops)
                #   nat = tcs + tsn  -> bf16    (gpsimd)
                qs = ropep.tile([128, 4, 128], F32, tag="qs")
                qsc_b = qsc[:, 0:4].unsqueeze(2).broadcast_to([128, 4, 128])
                q3 = q_ps[:].rearrange("p (h d) -> p h d", h=4)
                nc.vector.tensor_tensor(qs[:], q3, qsc_b, AluOpType.mult)
                cos_b = cos2[:, si, :].unsqueeze(1).broadcast_to([128, 4, 128])
                tcs = ropep.tile([128, 4, 128], F32, tag="tcs")
                nc.vector.tensor_tensor(tcs[:], qs[:], cos_b, AluOpType.mult)
                tsn = ropep.tile([128, 4, 128], F32, tag="tsn")
                sinA = sin2[:, si, 0:64].unsqueeze(1).broadcast_to([128, 4, 64])
                sinB = sin2[:, si, 64:128].unsqueeze(1).broadcast_to([128, 4, 64])
                nc.vector.tensor_tensor(tsn[:, :, 0:64], qs[:, :, 64:128], sinA,
                                        AluOpType.mult)
                nc.vector.tensor_tensor(tsn[:, :, 64:128], qs[:, :, 0:64], sinB,
                                        AluOpType.mult)
                natq = ropep.tile([128, 4, 128], BF16, tag="natq")
                nc.gpsimd.tensor_tensor(natq[:], tcs[:], tsn[:], AluOpType.add)

                # k rope (1 head): fused scale via scalar_tensor_tensor
                kcs = ropep.tile([128, 128], F32, tag="kcs")
                ksn = ropep.tile([128, 128], F32, tag="ksn")
                kraw = kv_ps[:, 0:128]
                nc.vector.scalar_tensor_tensor(
                    kcs[:], kraw, rn[:, 4:5], cos2[:, si, :], AluOpType.mult,
                    AluOpType.mult)
                nc.vector.scalar_tensor_tensor(
                    ksn[:, 0:64], kraw[:, 64:128], rn[:, 4:5], sin2[:, si, 0:64],
                    AluOpType.mult, AluOpType.mult)
                nc.vector.scalar_tensor_tensor(
                    ksn[:, 64:128], kraw[:, 0:64], rn[:, 4:5], sin2[:, si, 64:128],
                    AluOpType.mult, AluOpType.mult)
                natk = ropep.tile([128, 128], BF16, tag="natk")
                nc.gpsimd.tensor_tensor(natk[:], kcs[:], ksn[:], AluOpType.add)

                # v: ACT copy out of psum (fp32 -> bf16)
                nc.scalar.copy(v_nat[:, si, :], kv_ps[:, 128:256])

                # transposes delayed one s-tile so PE never waits on rope
                flush_tp()

                def make_tp(si=si, natq=natq, natk=natk):
                    def tp_():
                        for h in range(NH):
                            tp = psS.tile([128, 128], BF16, tag="S", name=f"tpq_{si}_{h}")
                            nc.tensor.transpose(tp[:], natq[:, h, :], ident[:])
                            nc.vector.tensor_copy(qT[:, h, si * 128:(si + 1) * 128], tp[:])
                        tp = psS.tile([128, 128], BF16, tag="S", name=f"tpk_{si}")
                        nc.tensor.transpose(tp[:], natk[:], ident[:])
                        nc.vector.tensor_copy(kT[:, si * 128:(si + 1) * 128], tp[:])
                    return tp_
                pending_tp[0] = make_tp()
            flush_tp()

            # ---------------- phase 2: attention, two heads interleaved,
            # software-pipelined: emit sc(j), exp(j), then pv/rs(j-1).
            pending_epilogue = [None]

            def flush_epilogue():
                if pending_epilogue[0] is not None:
                    pending_epilogue[0]()
                    pending_epilogue[0] = None

            for t in range(T):
                yt_t = ytp.tile([128, NH, 512], BF16, tag="yt")
                nblk = 4 * t + 4
                for hp in (0, 2):
                    o_ps = {}
                    rs_ps = {}
                    for h in (hp, hp + 1):
                        o_ps[h] = psA.tile([128, 512], F32, tag="A", name=f"o_ps_{t}_{h}")
                        rs_ps[h] = psB.tile([128, 512], F32, tag="B", name=f"rs_ps_{t}_{h}")

                    sc_tiles = {}
                    prb_tiles = {}

                    def emit_sc(j, t=t, hp=hp, sc_tiles=sc_tiles, prb_tiles=prb_tiles):
                        off = j - 4 * t
                        q0 = max(off, 0) * 128   # live q start within the slice
                        sc = psS.tile([128, 1024], F32, tag="S", name=f"sc_{t}_{hp}_{j}")
                        for u, h in enumerate((hp, hp + 1)):
                            nc.tensor.matmul(
                                sc[:, u * 512 + q0:(u + 1) * 512],
                                kT[:, j * 128:(j + 1) * 128],
                                qT[:, h, t * 512 + q0:(t + 1) * 512],
                                start=True, stop=True, skip_group_check=True)
                        prb = probsp.tile([128, 1024], BF16, tag="probs",
                                          name=f"prb_{t}_{hp}_{j}")
                        if off <= 0:
                            nc.scalar.activation(prb[:], sc[:], AF.Exp)
                        else:
                            for u in range(2):
                                nc.scalar.activation(
                                    prb[:, u * 512 + q0:(u + 1) * 512],
                                    sc[:, u * 512 + q0:(u + 1) * 512], AF.Exp)
                        if off >= 0:
                            # mask the 128-wide diagonal sub-block
                            for u in range(2):
                                nc.vector.tensor_tensor(
                                    prb[:, u * 512 + q0:u * 512 + q0 + 128],
                                    prb[:, u * 512 + q0:u * 512 + q0 + 128],
                                    tri[:], AluOpType.mult)
                        sc_tiles[j] = sc
                        prb_tiles[j] = prb

                    def emit_pvrs(j, t=t, hp=hp, o_ps=o_ps, rs_ps=rs_ps,
                                  prb_tiles=prb_tiles, nblk=nblk):
                        off = j - 4 * t
                        q0 = max(off, 0) * 128
                        prb = prb_tiles.pop(j)
                        first = (j == 0)
                        last = (j == nblk - 1)
                        for u, h in enumerate((hp, hp + 1)):
                            nc.tensor.matmul(
                                o_ps[h][:, q0:512], v_nat[:, j, :],
                                prb[:, u * 512 + q0:(u + 1) * 512],
                                start=first, stop=last, skip_group_check=True)
                            # all-ones stationary: row sums land pre-broadcast
                            # on all 128 partitions
                            nc.tensor.matmul(
                                rs_ps[h][:, q0:512], onesq[:],
                                prb[:, u * 512 + q0:(u + 1) * 512],
                                start=first, stop=last, skip_group_check=True)

                    emit_sc(0)
                    emit_sc(1)
                    for j in range(2, nblk):
                        emit_sc(j)
                        emit_pvrs(j - 2)
                        if j == 2:
                            flush_epilogue()
                    emit_pvrs(nblk - 2)
                    emit_pvrs(nblk - 1)

                    # normalize: rs is already broadcast across partitions by
                    # the all-ones matmul; fast approx reciprocal (18 bits,
                    # plenty for bf16 output) then one multiply
                    def make_epilogue(o_ps=o_ps, rs_ps=rs_ps, yt_t=yt_t, hp=hp):
                        def ep():
                            for h in (hp, hp + 1):
                                rcp = normp.tile([128, 512], F32, tag="rcp")
                                nc.vector.reciprocal_approx_fast(rcp[:], rs_ps[h][:])
                                nc.vector.tensor_tensor(
                                    yt_t[:, h, :], o_ps[h][:], rcp[:],
                                    AluOpType.mult)
                        return ep
                    pending_epilogue[0] = make_epilogue()

                # ---------------- phase 3 (per t): projection for s-tiles 4t..4t+3
                flush_epilogue()
                for si in range(4 * t, 4 * t + 4):
                    sl = si - 4 * t
                    for dtp in range(2):
                        pj0 = psB.tile([128, 512], F32, tag="B")
                        pj1 = psB.tile([128, 512], F32, tag="B")
                        for h in range(NH):
                            lhs = yt_t[:, h, sl * 128:(sl + 1) * 128]
                            nc.tensor.matmul(pj0[:], lhs, wp[2 * dtp][:, h, :],
                                             start=(h == 0), stop=(h == NH - 1),
                                             skip_group_check=True)
                            nc.tensor.matmul(pj1[:], lhs, wp[2 * dtp + 1][:, h, :],
                                             start=(h == 0), stop=(h == NH - 1),
                                             skip_group_check=True)
                        ev = outsb.tile([128, 1024], BF16, tag="ev")
                        nc.scalar.copy(ev[:, 0:512], pj0[:])
                        nc.scalar.copy(ev[:, 512:1024], pj1[:])
                        nc.sync.dma_start(
                            Y[si * 128:(si + 1) * 128,
                              dtp * 1024:(dtp + 1) * 1024], ev[:])

    nc.compile()
    return nc


def _host_inputs(x, Wq, Wk, Wv, Wproj, q_gain):
    import ml_dtypes
    bf16 = ml_dtypes.bfloat16

    x = np.asarray(x, dtype=np.float32)
    Wq = np.asarray(Wq, dtype=np.float32)
    Wk = np.asarray(Wk, dtype=np.float32)
    Wv = np.asarray(Wv, dtype=np.float32)
    Wproj = np.asarray(Wproj, dtype=np.float32)
    q_gain = np.asarray(q_gain, dtype=np.float32)

    inv = (1.0 / ROPE_BASE ** (np.arange(0, HD, 2, dtype=np.float32) / HD)).astype(np.float32)
    ang = np.outer(np.arange(S, dtype=np.float32), inv)
    cos = np.cos(ang).astype(np.float32)
    sin = np.sin(ang).astype(np.float32)
    cos2 = np.concatenate([cos, cos], 1).reshape(NT, 128, HD).transpose(1, 0, 2).copy()
    sin2 = np.concatenate([sin, -sin], 1).reshape(NT, 128, HD).transpose(1, 0, 2).copy()

    qq = np.arange(128)[None, :]
    kk = np.arange(128)[:, None]
    tri = (kk <= qq).astype(bf16)
    ident = np.eye(128, dtype=bf16)
    onesq = np.ones((128, 128), dtype=bf16)

    # pre-tile x[b].T as [si4, c4, p, a, s] (contiguous 4KB per partition line)
    xTb = [np.ascontiguousarray(
        x[b].T.astype(bf16).reshape(4, 4, 128, 4, 512).transpose(3, 0, 2, 1, 4))
        for b in range(B)]

    in_maps = []
    for cid in range(8):
        b, g = cid // 4, cid % 4
        wq = Wq[g * 512:(g + 1) * 512, :].T            # [D, 512]
        wk = Wk[g * 128:(g + 1) * 128, :].T            # [D, 128]
        wv = Wv[g * 128:(g + 1) * 128, :].T
        wkv = np.concatenate([wk, wv], 1)              # [D, 256]
        wp = Wproj[:, g * 512:(g + 1) * 512].T         # [512, D] (c_local, dout)
        in_maps.append({
            "XT": xTb[b],
            "WQ": np.ascontiguousarray(
                wq.reshape(NC_, 128, 512).transpose(1, 0, 2)).astype(bf16),
            "WKV": np.ascontiguousarray(
                wkv.reshape(NC_, 128, 256).transpose(1, 0, 2)).astype(bf16),
            "WP": np.ascontiguousarray(
                wp.reshape(NH, 128, 4, 512).transpose(2, 1, 0, 3)).astype(bf16),
            "COS2": cos2, "SIN2": sin2,
            "GSM": (q_gain[g * 4:(g + 1) * 4] * SM).reshape(1, NH).astype(np.float32),
            "IDENT": ident, "ONESQ": onesq, "TRI": tri,
        })
    return in_maps


def _get_prog():
    global _PROG
    if _PROG is None:
        _PROG = _build_program()
    return _PROG


def kernel(x, Wq, Wk, Wv, Wproj, q_gain, _trace=False, _tmpdir=None):
    from concourse.bass_utils import run_bass_kernel_spmd
    nc = _get_prog()
    in_maps = _host_inputs(x, Wq, Wk, Wv, Wproj, q_gain)
    kwargs = {}
    if _tmpdir is not None:
        os.makedirs(_tmpdir, exist_ok=True)
        kwargs["tmpdir"] = _tmpdir
    res = run_bass_kernel_spmd(nc, in_maps, list(range(8)), trace=_trace, **kwargs)
    y = np.empty((B, S, D), dtype=np.float32)
    for b in range(B):
        acc = res.results[4 * b]["Y"].astype(np.float32)
        for g in range(1, 4):
            acc = acc + res.results[4 * b + g]["Y"].astype(np.float32)
        y[b] = acc
    if _trace:
        kernel._last_result = res
    return y


# revision 25
# speedup vs baseline: 1.6485x; 1.0183x over previous
"""Causal GQA self-attention (B=2, S=2048, D=2048, H=16, KV=4) on 8 TRN2 cores.

Sharding: core = (b, g) with b = batch (2) x g = kv-head group (4).
Each core computes 4 q-heads / 1 kv-head for one batch and a partial
projection output [S, D] in bf16; host sums the 4 group partials per batch.

v2 changes vs baseline (496us):
  - all matmul operands bf16 (x, W, qT/kT/v, probs, yt, Wp); psum stays fp32.
  - epilogue: reciprocal on [1,512] BEFORE broadcast (was reciprocal of
    [128,512] = 56us DVE); chain is recip -> gpsimd bcast -> one DVE mult.
  - proj eviction via ACT copy (psum->bf16 sbuf) + dma; no DVE in proj path.
  - rope: batched ops across the 4 q heads via broadcast APs (3 DVE + 1
    gpsimd per s-tile instead of 12 DVE + 4 gpsimd).
  - causal diagonal trim: diag blocks only compute live q columns.
  - software-pipelined: transposes delayed one s-tile; attention exp/pv
    chain pipelined so PE never waits on ACT exp.
  - weight/x DMA split across 4 engine queues; first matmul after ~1.3MB.
"""
import os
import sys

if '/opt/trn_rl_repo' not in sys.path:
    sys.path.insert(0, '/opt/trn_rl_repo')

import numpy as np

B, S, D = 2, 2048, 2048
NH_TOT, NKV_TOT, HD = 16, 4, 128
NH = 4                 # q heads per core
NT = S // 128          # 16 s-tiles
NC_ = D // 128         # 16 c-tiles
T = 4                  # q-slices of 512
SM = 1.0 / np.sqrt(HD)
EPS = float(np.finfo(np.float32).eps)
ROPE_BASE = 10000.0

_PROG = None


def _build_program():
    import concourse.bass as bass
    import concourse.mybir as mybir
    import concourse.tile as tile
    from concourse import bacc
    from concourse.alu_op_type import AluOpType

    F32 = mybir.dt.float32
    BF16 = mybir.dt.bfloat16
    AF = mybir.ActivationFunctionType

    nc = bacc.Bacc("TRN2", target_bir_lowering=False, debug=False)

    # x[b].T pre-tiled: [si4, c4, p, a, s] so each (si4,c4) load is 128x4KB
    XT = nc.dram_tensor("XT", [4, 4, 128, 4, 512], BF16, kind="ExternalInput")
    WQ = nc.dram_tensor("WQ", [128, NC_, 512], BF16, kind="ExternalInput")   # Wq_g.T tiled [c_p, ci, dq]
    WKV = nc.dram_tensor("WKV", [128, NC_, 256], BF16, kind="ExternalInput")
    WP = nc.dram_tensor("WP", [4, 128, NH, 512], BF16, kind="ExternalInput")  # [dq, c_in_head, h, dout]
    COS2 = nc.dram_tensor("COS2", [128, NT, HD], F32, kind="ExternalInput")
    SIN2 = nc.dram_tensor("SIN2", [128, NT, HD], F32, kind="ExternalInput")
    GSM = nc.dram_tensor("GSM", [1, NH], F32, kind="ExternalInput")          # gain*sm per head
    IDENT = nc.dram_tensor("IDENT", [128, 128], BF16, kind="ExternalInput")
    ONESQ = nc.dram_tensor("ONESQ", [128, 128], BF16, kind="ExternalInput")
    TRI = nc.dram_tensor("TRI", [128, 128], BF16, kind="ExternalInput")      # lower-tri ones
    Y = nc.dram_tensor("Y", [S, D], BF16, kind="ExternalOutput")

    with tile.TileContext(nc) as tc:
        with (
            tc.tile_pool(name="const", bufs=1) as const,
            tc.tile_pool(name="w", bufs=4) as wpool,
            tc.tile_pool(name="stream", bufs=3) as stream,
            tc.tile_pool(name="small", bufs=3) as small,
            tc.tile_pool(name="norm", bufs=4) as normp,
            tc.tile_pool(name="rope", bufs=2) as ropep,
            tc.tile_pool(name="big", bufs=1) as big,
            tc.tile_pool(name="yt", bufs=2) as ytp,
            tc.tile_pool(name="probs", bufs=4) as probsp,
            tc.tile_pool(name="outsb", bufs=4) as outsb,
            tc.tile_pool(name="psA", bufs=2, space="PSUM") as psA,
            tc.tile_pool(name="psB", bufs=2, space="PSUM") as psB,
            tc.tile_pool(name="psS", bufs=2, space="PSUM") as psS,
        ):
            # --- startup DMA. Queues deliver ~0.1-0.13 MB/us each, so the
            # first s-tile is weight/x-DMA paced: deliver in fine pieces,
            # spread across the three DMA-capable queues, and consume c-tiles
            # in arrival order (ci_order below matches queue assignment).
            wqkv = [wpool.tile([128, 4, 768], BF16, tag="w", name=f"wqkv{i}")
                    for i in range(4)]
            xs4 = []
            xs0 = stream.tile([128, NC_, 512], BF16, tag="xs")
            # sync: xs0 c0 per-a pieces, xs0 c1, WQ3, xs0 c2, xs0 c3, WKV3
            for a in range(4):
                nc.sync.dma_start(xs0[:, a:a + 1, :], XT[0, 0][:, a:a + 1, :])
            nc.sync.dma_start(xs0[:, 4:8, :], XT[0, 1])
            nc.sync.dma_start(wqkv[3][:, :, 0:512], WQ[:, 12:16, :])
            nc.sync.dma_start(xs0[:, 8:12, :], XT[0, 2])
            nc.sync.dma_start(xs0[:, 12:16, :], XT[0, 3])
            nc.sync.dma_start(wqkv[3][:, :, 512:768], WKV[:, 12:16, :])
            xs4.append(xs0)

            gsm = const.tile([1, NH], F32)
            nc.scalar.dma_start(gsm[:], GSM[:])
            ident = const.tile([128, 128], BF16)
            nc.scalar.dma_start(ident[:], IDENT[:])
            onesq = const.tile([128, 128], BF16)
            nc.scalar.dma_start(onesq[:], ONESQ[:])
            tri = const.tile([128, 128], BF16)
            nc.scalar.dma_start(tri[:], TRI[:])
            # scalar: WQ0 per-ci, WQ1 per-ci, WKV0, WKV1, cos/sin
            for a in range(4):
                nc.scalar.dma_start(wqkv[0][:, a:a + 1, 0:512], WQ[:, a:a + 1, :])
            for a in range(4):
                nc.scalar.dma_start(wqkv[1][:, a:a + 1, 0:512], WQ[:, 4 + a:5 + a, :])
            nc.scalar.dma_start(wqkv[0][:, :, 512:768], WKV[:, 0:4, :])
            nc.scalar.dma_start(wqkv[1][:, :, 512:768], WKV[:, 4:8, :])
            cos2 = const.tile([128, NT, HD], F32)
            nc.scalar.dma_start(cos2[:], COS2[:])
            sin2 = const.tile([128, NT, HD], F32)
            nc.scalar.dma_start(sin2[:], SIN2[:])

            # gpsimd: WQ2, WKV2, proj weights
            nc.gpsimd.dma_start(wqkv[2][:, :, 0:512], WQ[:, 8:12, :])
            nc.gpsimd.dma_start(wqkv[2][:, :, 512:768], WKV[:, 8:12, :])
            gsm_bc = const.tile([128, NH], F32)
            nc.gpsimd.partition_broadcast(gsm_bc[:], gsm[:])
            wp = []
            for dq in range(4):
                wt = wpool.tile([128, NH, 512], BF16, tag="wp")
                nc.gpsimd.dma_start(wt[:], WP[dq])
                wp.append(wt)

            # consume c-tiles in DMA arrival order: WQ0 (scalar), WQ2
            # (gpsimd), WQ1 (scalar), WQ3 (sync, behind xs0)
            ci_order = [0, 1, 2, 3, 8, 9, 10, 11, 4, 5, 6, 7, 12, 13, 14, 15]

            qT = big.tile([128, NH, S], BF16)
            kT = big.tile([128, S], BF16)
            v_nat = big.tile([128, NT, HD], BF16)

            # ---------------- phase 1: QKV + rms-norm + rope + transpose
            pending_tp = [None]

            def flush_tp():
                if pending_tp[0] is not None:
                    pending_tp[0]()
                    pending_tp[0] = None

            for si in range(NT):
                c4i = si // 4
                sl = si % 4
                while sl == 0 and len(xs4) < min(4, c4i + 3):
                    nxt = stream.tile([128, NC_, 512], BF16, tag="xs",
                                      name=f"xs{len(xs4)}")
                    s4 = len(xs4)
                    for c4 in range(4):
                        nc.sync.dma_start(nxt[:, 4 * c4:4 * c4 + 4, :],
                                          XT[s4, c4])
                    xs4.append(nxt)
                xs = xs4[c4i]
                q_ps = psA.tile([128, 512], F32, tag="A")
                kv_ps = psB.tile([128, 256], F32, tag="B")
                for i, ci in enumerate(ci_order):
                    nc.tensor.matmul(q_ps[:], xs[:, ci, sl * 128:(sl + 1) * 128],
                                     wqkv[ci // 4][:, ci % 4, 0:512],
                                     start=(i == 0), stop=(i == NC_ - 1))
                for i, ci in enumerate(ci_order):
                    nc.tensor.matmul(kv_ps[:], xs[:, ci, sl * 128:(sl + 1) * 128],
                                     wqkv[ci // 4][:, ci % 4, 512:768],
                                     start=(i == 0), stop=(i == NC_ - 1))

                # sum of squares per head (q: 4 heads, k: 1) on ACT
                scr = small.tile([128, 128], F32, tag="scr")
                ssq = small.tile([128, 8], F32, tag="ssq")
                for h in range(NH):
                    nc.scalar.activation(scr[:], q_ps[:, h * 128:(h + 1) * 128],
                                         AF.Square, accum_out=ssq[:, h:h + 1])
                nc.scalar.activation(scr[:], kv_ps[:, 0:128], AF.Square,
                                     accum_out=ssq[:, 4:5])
                mn = small.tile([128, 8], F32, tag="mn")
                nc.vector.tensor_scalar(mn[:, 0:5], ssq[:, 0:5], 1.0 / HD, EPS,
                                        AluOpType.mult, AluOpType.add)
                rt = small.tile([128, 8], F32, tag="rt")
                nc.scalar.sqrt(rt[:, 0:5], mn[:, 0:5])
                rn = small.tile([128, 8], F32, tag="rn")
                nc.vector.reciprocal(rn[:, 0:5], rt[:, 0:5])
                qsc = small.tile([128, 4], F32, tag="qsc")
                nc.vector.tensor_tensor(qsc[:], rn[:, 0:4], gsm_bc[:], AluOpType.mult)

                # batched rope for the 4 q heads:
                #   qs  = q_ps * qsc[head]      (scale, per-head broadcast AP)
                #   tcs = qs * cos[rep]         (full width)
                #   tsn = swap_halves(qs) * sin[rep]  (two half ops)
                #   nat = tcs + tsn  -> bf16    (gpsimd)
                qs = ropep.tile([128, 4, 128], F32, tag="qs")
                qsc_b = qsc[:, 0:4].unsqueeze(2).broadcast_to([128, 4, 128])
                q3 = q_ps[:].rearrange("p (h d) -> p h d", h=4)
                nc.vector.tensor_tensor(qs[:], q3, qsc_b, AluOpType.mult)
                cos_b = cos2[:, si, :].unsqueeze(1).broadcast_to([128, 4, 128])
                tcs = ropep.tile([128, 4, 128], F32, tag="tcs")
                nc.vector.tensor_tensor(tcs[:], qs[:], cos_b, AluOpType.mult)
                tsn = ropep.tile([128, 4, 128], F32, tag="tsn")
                sinA = sin2[:, si, 0:64].unsqueeze(1).broadcast_to([128, 4, 64])
                sinB = sin2[:, si, 64:128].unsqueeze(1).broadcast_to([128, 4, 64])
                nc.vector.tensor_tensor(tsn[:, :, 0:64], qs[:, :, 64:128], sinA,
                                        AluOpType.mult)
                nc.vector.tensor_tensor(tsn[:, :, 64:128], qs[:, :, 0:64], sinB,
                                        AluOpType.mult)
                natq = ropep.tile([128, 4, 128], BF16, tag="natq")
                nc.gpsimd.tensor_tensor(natq[:], tcs[:], tsn[:], AluOpType.add)

                # k rope (1 head): fused scale via scalar_tensor_tensor
                kcs = ropep.tile([128, 128], F32, tag="kcs")
                ksn = ropep.tile([128, 128], F32, tag="ksn")
                kraw = kv_ps[:, 0:128]
                nc.vector.scalar_tensor_tensor(
                    kcs[:], kraw, rn[:, 4:5], cos2[:, si, :], AluOpType.mult,
                    AluOpType.mult)
                nc.vector.scalar_tensor_tensor(
                    ksn[:, 0:64], kraw[:, 64:128], rn[:, 4:5], sin2[:, si, 0:64],
                    AluOpType.mult, AluOpType.mult)
                nc.vector.scalar_tensor_tensor(
                    ksn[:, 64:128], kraw[:, 0:64], rn[:, 4:5], sin2[:, si, 64:128],
                    AluOpType.mult, AluOpType.mult)
                natk = ropep.tile([128, 128], BF16, tag="natk")
                nc.gpsimd.tensor_tensor(natk[:], kcs[:], ksn[:], AluOpType.add)

                # v: ACT copy out of psum (fp32 -> bf16)
                nc.scalar.copy(v_nat[:, si, :], kv_ps[:, 128:256])

                # transposes delayed one s-tile so PE never waits on rope
                flush_tp()

                def make_tp(si=si, natq=natq, natk=natk):
                    def tp_():
                        for h in range(NH):
                            tp = psS.tile([128, 128], BF16, tag="S", name=f"tpq_{si}_{h}")
                            nc.tensor.transpose(tp[:], natq[:, h, :], ident[:])
                            nc.vector.tensor_copy(qT[:, h, si * 128:(si + 1) * 128], tp[:])
                        tp = psS.tile([128, 128], BF16, tag="S", name=f"tpk_{si}")
                        nc.tensor.transpose(tp[:], natk[:], ident[:])
                        nc.vector.tensor_copy(kT[:, si * 128:(si + 1) * 128], tp[:])
                    return tp_
                pending_tp[0] = make_tp()
            # last s-tile's transposes are flushed after the first attention
            # unit (they are first needed by t=3), hiding the rope tail

            # ---------------- phase 2: attention, two heads interleaved,
            # software-pipelined: emit sc(j), exp(j), then pv/rs(j-1).
            pending_epilogue = [None]

            def flush_epilogue():
                if pending_epilogue[0] is not None:
                    pending_epilogue[0]()
                    pending_epilogue[0] = None

            for t in range(T):
                yt_t = ytp.tile([128, NH, 512], BF16, tag="yt")
                nblk = 4 * t + 4
                for hp in (0, 2):
                    o_ps = {}
                    rs_ps = {}
                    for h in (hp, hp + 1):
                        o_ps[h] = psA.tile([128, 512], F32, tag="A", name=f"o_ps_{t}_{h}")
                        rs_ps[h] = psB.tile([128, 512], F32, tag="B", name=f"rs_ps_{t}_{h}")

                    sc_tiles = {}
                    prb_tiles = {}

                    def emit_sc(j, t=t, hp=hp, sc_tiles=sc_tiles, prb_tiles=prb_tiles):
                        off = j - 4 * t
                        q0 = max(off, 0) * 128   # live q start within the slice
                        sc = psS.tile([128, 1024], F32, tag="S", name=f"sc_{t}_{hp}_{j}")
                        for u, h in enumerate((hp, hp + 1)):
                            nc.tensor.matmul(
                                sc[:, u * 512 + q0:(u + 1) * 512],
                                kT[:, j * 128:(j + 1) * 128],
                                qT[:, h, t * 512 + q0:(t + 1) * 512],
                                start=True, stop=True, skip_group_check=True)
                        prb = probsp.tile([128, 1024], BF16, tag="probs",
                                          name=f"prb_{t}_{hp}_{j}")
                        if off <= 0:
                            nc.scalar.activation(prb[:], sc[:], AF.Exp)
                        else:
                            for u in range(2):
                                nc.scalar.activation(
                                    prb[:, u * 512 + q0:(u + 1) * 512],
                                    sc[:, u * 512 + q0:(u + 1) * 512], AF.Exp)
                        if off >= 0:
                            # mask the 128-wide diagonal sub-block
                            for u in range(2):
                                nc.vector.tensor_tensor(
                                    prb[:, u * 512 + q0:u * 512 + q0 + 128],
                                    prb[:, u * 512 + q0:u * 512 + q0 + 128],
                                    tri[:], AluOpType.mult)
                        sc_tiles[j] = sc
                        prb_tiles[j] = prb

                    def emit_pvrs(j, t=t, hp=hp, o_ps=o_ps, rs_ps=rs_ps,
                                  prb_tiles=prb_tiles, nblk=nblk):
                        off = j - 4 * t
                        q0 = max(off, 0) * 128
                        prb = prb_tiles.pop(j)
                        first = (j == 0)
                        last = (j == nblk - 1)
                        for u, h in enumerate((hp, hp + 1)):
                            nc.tensor.matmul(
                                o_ps[h][:, q0:512], v_nat[:, j, :],
                                prb[:, u * 512 + q0:(u + 1) * 512],
                                start=first, stop=last, skip_group_check=True)
                            # all-ones stationary: row sums land pre-broadcast
                            # on all 128 partitions
                            nc.tensor.matmul(
                                rs_ps[h][:, q0:512], onesq[:],
                                prb[:, u * 512 + q0:(u + 1) * 512],
                                start=first, stop=last, skip_group_check=True)

                    depth = 3
                    for j in range(depth):
                        emit_sc(j)
                    for j in range(depth, nblk):
                        emit_sc(j)
                        emit_pvrs(j - depth)
                        if j == depth:
                            flush_epilogue()
                    for j in range(nblk - depth, nblk):
                        emit_pvrs(j)
                        if nblk == depth and j == nblk - depth:
                            flush_epilogue()
                    if t == 0 and hp == 0:
                        flush_tp()

                    # normalize: rs is already broadcast across partitions by
                    # the all-ones matmul; fast approx reciprocal (18 bits,
                    # plenty for bf16 output) then one multiply
                    def make_epilogue(o_ps=o_ps, rs_ps=rs_ps, yt_t=yt_t, hp=hp):
                        def ep():
                            for h in (hp, hp + 1):
                                rcp = normp.tile([128, 512], F32, tag="rcp")
                                nc.vector.reciprocal_approx_fast(rcp[:], rs_ps[h][:])
                                nc.vector.tensor_tensor(
                                    yt_t[:, h, :], o_ps[h][:], rcp[:],
                                    AluOpType.mult)
                        return ep
                    pending_epilogue[0] = make_epilogue()

                # ---------------- phase 3 (per t): projection for s-tiles 4t..4t+3
                flush_epilogue()
                for si in range(4 * t, 4 * t + 4):
                    sl = si - 4 * t
                    for dtp in range(2):
                        # pj0 borrows the attention-score psum pool (idle
                        # during proj) so consecutive groups double-buffer
                        pj0 = psS.tile([128, 512], F32, tag="S")
                        pj1 = psB.tile([128, 512], F32, tag="B")
                        for h in range(NH):
                            lhs = yt_t[:, h, sl * 128:(sl + 1) * 128]
                            nc.tensor.matmul(pj0[:], lhs, wp[2 * dtp][:, h, :],
                                             start=(h == 0), stop=(h == NH - 1),
                                             skip_group_check=True)
                            nc.tensor.matmul(pj1[:], lhs, wp[2 * dtp + 1][:, h, :],
                                             start=(h == 0), stop=(h == NH - 1),
                                             skip_group_check=True)
                        ev = outsb.tile([128, 1024], BF16, tag="ev")
                        nc.scalar.copy(ev[:, 0:512], pj0[:])
                        nc.vector.tensor_copy(ev[:, 512:1024], pj1[:])
                        nc.sync.dma_start(
                            Y[si * 128:(si + 1) * 128,
                              dtp * 1024:(dtp + 1) * 1024], ev[:])

    nc.compile()
    return nc


def _host_inputs(x, Wq, Wk, Wv, Wproj, q_gain):
    import ml_dtypes
    bf16 = ml_dtypes.bfloat16

    x = np.asarray(x, dtype=np.float32)
    Wq = np.asarray(Wq, dtype=np.float32)
    Wk = np.asarray(Wk, dtype=np.float32)
    Wv = np.asarray(Wv, dtype=np.float32)
    Wproj = np.asarray(Wproj, dtype=np.float32)
    q_gain = np.asarray(q_gain, dtype=np.float32)

    inv = (1.0 / ROPE_BASE ** (np.arange(0, HD, 2, dtype=np.float32) / HD)).astype(np.float32)
    ang = np.outer(np.arange(S, dtype=np.float32), inv)
    cos = np.cos(ang).astype(np.float32)
    sin = np.sin(ang).astype(np.float32)
    cos2 = np.concatenate([cos, cos], 1).reshape(NT, 128, HD).transpose(1, 0, 2).copy()
    sin2 = np.concatenate([sin, -sin], 1).reshape(NT, 128, HD).transpose(1, 0, 2).copy()

    qq = np.arange(128)[None, :]
    kk = np.arange(128)[:, None]
    tri = (kk <= qq).astype(bf16)
    ident = np.eye(128, dtype=bf16)
    onesq = np.ones((128, 128), dtype=bf16)

    # pre-tile x[b].T as [si4, c4, p, a, s] (contiguous 4KB per partition line)
    xTb = [np.ascontiguousarray(
        x[b].T.astype(bf16).reshape(4, 4, 128, 4, 512).transpose(3, 0, 2, 1, 4))
        for b in range(B)]

    in_maps = []
    for cid in range(8):
        b, g = cid // 4, cid % 4
        wq = Wq[g * 512:(g + 1) * 512, :].T            # [D, 512]
        wk = Wk[g * 128:(g + 1) * 128, :].T            # [D, 128]
        wv = Wv[g * 128:(g + 1) * 128, :].T
        wkv = np.concatenate([wk, wv], 1)              # [D, 256]
        wp = Wproj[:, g * 512:(g + 1) * 512].T         # [512, D] (c_local, dout)
        in_maps.append({
            "XT": xTb[b],
            "WQ": np.ascontiguousarray(
                wq.reshape(NC_, 128, 512).transpose(1, 0, 2)).astype(bf16),
            "WKV": np.ascontiguousarray(
                wkv.reshape(NC_, 128, 256).transpose(1, 0, 2)).astype(bf16),
            "WP": np.ascontiguousarray(
                wp.reshape(NH, 128, 4, 512).transpose(2, 1, 0, 3)).astype(bf16),
            "COS2": cos2, "SIN2": sin2,
            "GSM": (q_gain[g * 4:(g + 1) * 4] * SM).reshape(1, NH).astype(np.float32),
            "IDENT": ident, "ONESQ": onesq, "TRI": tri,
        })
    return in_maps


def _get_prog():
    global _PROG
    if _PROG is None:
        _PROG = _build_program()
    return _PROG


def kernel(x, Wq, Wk, Wv, Wproj, q_gain, _trace=False, _tmpdir=None):
    from concourse.bass_utils import run_bass_kernel_spmd
    nc = _get_prog()
    in_maps = _host_inputs(x, Wq, Wk, Wv, Wproj, q_gain)
    kwargs = {}
    if _tmpdir is not None:
        os.makedirs(_tmpdir, exist_ok=True)
        kwargs["tmpdir"] = _tmpdir
    res = run_bass_kernel_spmd(nc, in_maps, list(range(8)), trace=_trace, **kwargs)
    y = np.empty((B, S, D), dtype=np.float32)
    for b in range(B):
        acc = res.results[4 * b]["Y"].astype(np.float32)
        for g in range(1, 4):
            acc = acc + res.results[4 * b + g]["Y"].astype(np.float32)
        y[b] = acc
    if _trace:
        kernel._last_result = res
    return y


# revision 26
# speedup vs baseline: 1.6616x; 1.0080x over previous
"""Causal GQA self-attention (B=2, S=2048, D=2048, H=16, KV=4) on 8 TRN2 cores.

Sharding: core = (b, g) with b = batch (2) x g = kv-head group (4).
Each core computes 4 q-heads / 1 kv-head for one batch and a partial
projection output [S, D] in bf16; host sums the 4 group partials per batch.

v2 changes vs baseline (496us):
  - all matmul operands bf16 (x, W, qT/kT/v, probs, yt, Wp); psum stays fp32.
  - epilogue: reciprocal on [1,512] BEFORE broadcast (was reciprocal of
    [128,512] = 56us DVE); chain is recip -> gpsimd bcast -> one DVE mult.
  - proj eviction via ACT copy (psum->bf16 sbuf) + dma; no DVE in proj path.
  - rope: batched ops across the 4 q heads via broadcast APs (3 DVE + 1
    gpsimd per s-tile instead of 12 DVE + 4 gpsimd).
  - causal diagonal trim: diag blocks only compute live q columns.
  - software-pipelined: transposes delayed one s-tile; attention exp/pv
    chain pipelined so PE never waits on ACT exp.
  - weight/x DMA split across 4 engine queues; first matmul after ~1.3MB.
"""
import os
import sys

if '/opt/trn_rl_repo' not in sys.path:
    sys.path.insert(0, '/opt/trn_rl_repo')

import numpy as np

B, S, D = 2, 2048, 2048
NH_TOT, NKV_TOT, HD = 16, 4, 128
NH = 4                 # q heads per core
NT = S // 128          # 16 s-tiles
NC_ = D // 128         # 16 c-tiles
T = 4                  # q-slices of 512
SM = 1.0 / np.sqrt(HD)
EPS = float(np.finfo(np.float32).eps)
ROPE_BASE = 10000.0

_PROG = None


def _build_program():
    import concourse.bass as bass
    import concourse.mybir as mybir
    import concourse.tile as tile
    from concourse import bacc
    from concourse.alu_op_type import AluOpType

    F32 = mybir.dt.float32
    BF16 = mybir.dt.bfloat16
    AF = mybir.ActivationFunctionType

    nc = bacc.Bacc("TRN2", target_bir_lowering=False, debug=False)

    # x[b].T pre-tiled: [si4, c4, p, a, s] so each (si4,c4) load is 128x4KB
    XT = nc.dram_tensor("XT", [4, 4, 128, 4, 512], BF16, kind="ExternalInput")
    WQ = nc.dram_tensor("WQ", [128, NC_, 512], BF16, kind="ExternalInput")   # Wq_g.T tiled [c_p, ci, dq]
    WKV = nc.dram_tensor("WKV", [128, NC_, 256], BF16, kind="ExternalInput")
    WP = nc.dram_tensor("WP", [4, 128, NH, 512], BF16, kind="ExternalInput")  # [dq, c_in_head, h, dout]
    COS2 = nc.dram_tensor("COS2", [128, NT, HD], F32, kind="ExternalInput")
    SIN2 = nc.dram_tensor("SIN2", [128, NT, HD], F32, kind="ExternalInput")
    GSM = nc.dram_tensor("GSM", [1, NH], F32, kind="ExternalInput")          # gain*sm per head
    IDENT = nc.dram_tensor("IDENT", [128, 128], BF16, kind="ExternalInput")
    ONESQ = nc.dram_tensor("ONESQ", [128, 128], BF16, kind="ExternalInput")
    TRI = nc.dram_tensor("TRI", [128, 128], BF16, kind="ExternalInput")      # lower-tri ones
    Y = nc.dram_tensor("Y", [S, D], BF16, kind="ExternalOutput")

    with tile.TileContext(nc) as tc:
        with (
            tc.tile_pool(name="const", bufs=1) as const,
            tc.tile_pool(name="w", bufs=4) as wpool,
            tc.tile_pool(name="stream", bufs=3) as stream,
            tc.tile_pool(name="small", bufs=3) as small,
            tc.tile_pool(name="norm", bufs=4) as normp,
            tc.tile_pool(name="rope", bufs=2) as ropep,
            tc.tile_pool(name="big", bufs=1) as big,
            tc.tile_pool(name="yt", bufs=2) as ytp,
            tc.tile_pool(name="probs", bufs=4) as probsp,
            tc.tile_pool(name="outsb", bufs=4) as outsb,
            tc.tile_pool(name="psA", bufs=2, space="PSUM") as psA,
            tc.tile_pool(name="psB", bufs=2, space="PSUM") as psB,
            tc.tile_pool(name="psS", bufs=2, space="PSUM") as psS,
        ):
            # --- startup DMA. Queues deliver ~0.1-0.13 MB/us each, so the
            # first s-tile is weight/x-DMA paced: deliver in fine pieces,
            # spread across the three DMA-capable queues, and consume c-tiles
            # in arrival order (ci_order below matches queue assignment).
            wqkv = [wpool.tile([128, 4, 768], BF16, tag="w", name=f"wqkv{i}")
                    for i in range(4)]
            xs4 = []
            xs0 = stream.tile([128, NC_, 512], BF16, tag="xs")
            # sync: xs0 in 512KB c-chunks, then WQ3/WKV3
            for c4 in range(4):
                nc.sync.dma_start(xs0[:, 4 * c4:4 * c4 + 4, :], XT[0, c4])
            nc.sync.dma_start(wqkv[3][:, :, 0:512], WQ[:, 12:16, :])
            nc.sync.dma_start(wqkv[3][:, :, 512:768], WKV[:, 12:16, :])
            xs4.append(xs0)

            gsm = const.tile([1, NH], F32)
            nc.scalar.dma_start(gsm[:], GSM[:])
            ident = const.tile([128, 128], BF16)
            nc.scalar.dma_start(ident[:], IDENT[:])
            onesq = const.tile([128, 128], BF16)
            nc.scalar.dma_start(onesq[:], ONESQ[:])
            tri = const.tile([128, 128], BF16)
            nc.scalar.dma_start(tri[:], TRI[:])
            # scalar: WQ0, WQ1, WKV0, WKV1, cos/sin
            nc.scalar.dma_start(wqkv[0][:, :, 0:512], WQ[:, 0:4, :])
            nc.scalar.dma_start(wqkv[1][:, :, 0:512], WQ[:, 4:8, :])
            nc.scalar.dma_start(wqkv[0][:, :, 512:768], WKV[:, 0:4, :])
            nc.scalar.dma_start(wqkv[1][:, :, 512:768], WKV[:, 4:8, :])
            cos2 = const.tile([128, NT, HD], F32)
            nc.scalar.dma_start(cos2[:], COS2[:])
            sin2 = const.tile([128, NT, HD], F32)
            nc.scalar.dma_start(sin2[:], SIN2[:])

            # gpsimd: WQ2, WKV2, proj weights
            nc.gpsimd.dma_start(wqkv[2][:, :, 0:512], WQ[:, 8:12, :])
            nc.gpsimd.dma_start(wqkv[2][:, :, 512:768], WKV[:, 8:12, :])
            gsm_bc = const.tile([128, NH], F32)
            nc.gpsimd.partition_broadcast(gsm_bc[:], gsm[:])
            wp = []
            for dq in range(4):
                wt = wpool.tile([128, NH, 512], BF16, tag="wp")
                nc.gpsimd.dma_start(wt[:], WP[dq])
                wp.append(wt)

            # consume c-tiles roughly in DMA arrival order
            ci_order = [0, 1, 2, 3, 8, 9, 10, 11, 4, 5, 6, 7, 12, 13, 14, 15]

            qT = big.tile([128, NH, S], BF16)
            kT = big.tile([128, S], BF16)
            v_nat = big.tile([128, NT, HD], BF16)

            # ---------------- phase 1: QKV + rms-norm + rope + transpose
            pending_tp = [None]

            def flush_tp():
                if pending_tp[0] is not None:
                    pending_tp[0]()
                    pending_tp[0] = None

            for si in range(NT):
                c4i = si // 4
                sl = si % 4
                while sl == 0 and len(xs4) < min(4, c4i + 3):
                    nxt = stream.tile([128, NC_, 512], BF16, tag="xs",
                                      name=f"xs{len(xs4)}")
                    s4 = len(xs4)
                    for c4 in range(4):
                        nc.sync.dma_start(nxt[:, 4 * c4:4 * c4 + 4, :],
                                          XT[s4, c4])
                    xs4.append(nxt)
                xs = xs4[c4i]
                q_ps = psA.tile([128, 512], F32, tag="A")
                kv_ps = psB.tile([128, 256], F32, tag="B")
                for i, ci in enumerate(ci_order):
                    nc.tensor.matmul(q_ps[:], xs[:, ci, sl * 128:(sl + 1) * 128],
                                     wqkv[ci // 4][:, ci % 4, 0:512],
                                     start=(i == 0), stop=(i == NC_ - 1))
                for i, ci in enumerate(ci_order):
                    nc.tensor.matmul(kv_ps[:], xs[:, ci, sl * 128:(sl + 1) * 128],
                                     wqkv[ci // 4][:, ci % 4, 512:768],
                                     start=(i == 0), stop=(i == NC_ - 1))

                # sum of squares per head (q: 4 heads, k: 1) on ACT
                scr = small.tile([128, 128], F32, tag="scr")
                ssq = small.tile([128, 8], F32, tag="ssq")
                for h in range(NH):
                    nc.scalar.activation(scr[:], q_ps[:, h * 128:(h + 1) * 128],
                                         AF.Square, accum_out=ssq[:, h:h + 1])
                nc.scalar.activation(scr[:], kv_ps[:, 0:128], AF.Square,
                                     accum_out=ssq[:, 4:5])
                mn = small.tile([128, 8], F32, tag="mn")
                nc.vector.tensor_scalar(mn[:, 0:5], ssq[:, 0:5], 1.0 / HD, EPS,
                                        AluOpType.mult, AluOpType.add)
                rt = small.tile([128, 8], F32, tag="rt")
                nc.scalar.sqrt(rt[:, 0:5], mn[:, 0:5])
                rn = small.tile([128, 8], F32, tag="rn")
                nc.vector.reciprocal(rn[:, 0:5], rt[:, 0:5])
                qsc = small.tile([128, 4], F32, tag="qsc")
                nc.vector.tensor_tensor(qsc[:], rn[:, 0:4], gsm_bc[:], AluOpType.mult)

                # batched rope for the 4 q heads:
                #   qs  = q_ps * qsc[head]      (scale, per-head broadcast AP)
                #   tcs = qs * cos[rep]         (full width)
                #   tsn = swap_halves(qs) * sin[rep]  (two half ops)
                #   nat = tcs + tsn  -> bf16    (gpsimd)
                qs = ropep.tile([128, 4, 128], F32, tag="qs")
                qsc_b = qsc[:, 0:4].unsqueeze(2).broadcast_to([128, 4, 128])
                q3 = q_ps[:].rearrange("p (h d) -> p h d", h=4)
                nc.vector.tensor_tensor(qs[:], q3, qsc_b, AluOpType.mult)
                cos_b = cos2[:, si, :].unsqueeze(1).broadcast_to([128, 4, 128])
                tcs = ropep.tile([128, 4, 128], F32, tag="tcs")
                nc.vector.tensor_tensor(tcs[:], qs[:], cos_b, AluOpType.mult)
                tsn = ropep.tile([128, 4, 128], F32, tag="tsn")
                sinA = sin2[:, si, 0:64].unsqueeze(1).broadcast_to([128, 4, 64])
                sinB = sin2[:, si, 64:128].unsqueeze(1).broadcast_to([128, 4, 64])
                nc.vector.tensor_tensor(tsn[:, :, 0:64], qs[:, :, 64:128], sinA,
                                        AluOpType.mult)
                nc.vector.tensor_tensor(tsn[:, :, 64:128], qs[:, :, 0:64], sinB,
                                        AluOpType.mult)
                natq = ropep.tile([128, 4, 128], BF16, tag="natq")
                nc.gpsimd.tensor_tensor(natq[:], tcs[:], tsn[:], AluOpType.add)

                # k rope (1 head): fused scale via scalar_tensor_tensor
                kcs = ropep.tile([128, 128], F32, tag="kcs")
                ksn = ropep.tile([128, 128], F32, tag="ksn")
                kraw = kv_ps[:, 0:128]
                nc.vector.scalar_tensor_tensor(
                    kcs[:], kraw, rn[:, 4:5], cos2[:, si, :], AluOpType.mult,
                    AluOpType.mult)
                nc.vector.scalar_tensor_tensor(
                    ksn[:, 0:64], kraw[:, 64:128], rn[:, 4:5], sin2[:, si, 0:64],
                    AluOpType.mult, AluOpType.mult)
                nc.vector.scalar_tensor_tensor(
                    ksn[:, 64:128], kraw[:, 0:64], rn[:, 4:5], sin2[:, si, 64:128],
                    AluOpType.mult, AluOpType.mult)
                natk = ropep.tile([128, 128], BF16, tag="natk")
                nc.gpsimd.tensor_tensor(natk[:], kcs[:], ksn[:], AluOpType.add)

                # v: ACT copy out of psum (fp32 -> bf16)
                nc.scalar.copy(v_nat[:, si, :], kv_ps[:, 128:256])

                # transposes delayed one s-tile so PE never waits on rope
                flush_tp()

                def make_tp(si=si, natq=natq, natk=natk):
                    def tp_():
                        for h in range(NH):
                            tp = psS.tile([128, 128], BF16, tag="S", name=f"tpq_{si}_{h}")
                            nc.tensor.transpose(tp[:], natq[:, h, :], ident[:])
                            nc.vector.tensor_copy(qT[:, h, si * 128:(si + 1) * 128], tp[:])
                        tp = psS.tile([128, 128], BF16, tag="S", name=f"tpk_{si}")
                        nc.tensor.transpose(tp[:], natk[:], ident[:])
                        nc.vector.tensor_copy(kT[:, si * 128:(si + 1) * 128], tp[:])
                    return tp_
                pending_tp[0] = make_tp()
            # last s-tile's transposes are flushed after the first attention
            # unit (they are first needed by t=3), hiding the rope tail

            # ---------------- phase 2: attention, two heads interleaved,
            # software-pipelined: emit sc(j), exp(j), then pv/rs(j-1).
            pending_epilogue = [None]

            def flush_epilogue():
                if pending_epilogue[0] is not None:
                    pending_epilogue[0]()
                    pending_epilogue[0] = None

            for t in range(T):
                yt_t = ytp.tile([128, NH, 512], BF16, tag="yt")
                nblk = 4 * t + 4
                for hp in (0, 2):
                    o_ps = {}
                    rs_ps = {}
                    for h in (hp, hp + 1):
                        o_ps[h] = psA.tile([128, 512], F32, tag="A", name=f"o_ps_{t}_{h}")
                        rs_ps[h] = psB.tile([128, 512], F32, tag="B", name=f"rs_ps_{t}_{h}")

                    sc_tiles = {}
                    prb_tiles = {}

                    def emit_sc(j, t=t, hp=hp, sc_tiles=sc_tiles, prb_tiles=prb_tiles):
                        off = j - 4 * t
                        q0 = max(off, 0) * 128   # live q start within the slice
                        sc = psS.tile([128, 1024], F32, tag="S", name=f"sc_{t}_{hp}_{j}")
                        for u, h in enumerate((hp, hp + 1)):
                            nc.tensor.matmul(
                                sc[:, u * 512 + q0:(u + 1) * 512],
                                kT[:, j * 128:(j + 1) * 128],
                                qT[:, h, t * 512 + q0:(t + 1) * 512],
                                start=True, stop=True, skip_group_check=True)
                        prb = probsp.tile([128, 1024], BF16, tag="probs",
                                          name=f"prb_{t}_{hp}_{j}")
                        if off <= 0:
                            nc.scalar.activation(prb[:], sc[:], AF.Exp)
                        else:
                            for u in range(2):
                                nc.scalar.activation(
                                    prb[:, u * 512 + q0:(u + 1) * 512],
                                    sc[:, u * 512 + q0:(u + 1) * 512], AF.Exp)
                        if off >= 0:
                            # mask the 128-wide diagonal sub-block
                            for u in range(2):
                                nc.vector.tensor_tensor(
                                    prb[:, u * 512 + q0:u * 512 + q0 + 128],
                                    prb[:, u * 512 + q0:u * 512 + q0 + 128],
                                    tri[:], AluOpType.mult)
                        sc_tiles[j] = sc
                        prb_tiles[j] = prb

                    def emit_pvrs(j, t=t, hp=hp, o_ps=o_ps, rs_ps=rs_ps,
                                  prb_tiles=prb_tiles, nblk=nblk):
                        off = j - 4 * t
                        q0 = max(off, 0) * 128
                        prb = prb_tiles.pop(j)
                        first = (j == 0)
                        last = (j == nblk - 1)
                        for u, h in enumerate((hp, hp + 1)):
                            nc.tensor.matmul(
                                o_ps[h][:, q0:512], v_nat[:, j, :],
                                prb[:, u * 512 + q0:(u + 1) * 512],
                                start=first, stop=last, skip_group_check=True)
                            # all-ones stationary: row sums land pre-broadcast
                            # on all 128 partitions
                            nc.tensor.matmul(
                                rs_ps[h][:, q0:512], onesq[:],
                                prb[:, u * 512 + q0:(u + 1) * 512],
                                start=first, stop=last, skip_group_check=True)

                    depth = 3
                    for j in range(depth):
                        emit_sc(j)
                    for j in range(depth, nblk):
                        emit_sc(j)
                        emit_pvrs(j - depth)
                        if j == depth:
                            flush_epilogue()
                    for j in range(nblk - depth, nblk):
                        emit_pvrs(j)
                        if nblk == depth and j == nblk - depth:
                            flush_epilogue()
                    if t == 0 and hp == 0:
                        flush_tp()

                    # normalize: rs is already broadcast across partitions by
                    # the all-ones matmul; fast approx reciprocal (18 bits,
                    # plenty for bf16 output) then one multiply
                    def make_epilogue(o_ps=o_ps, rs_ps=rs_ps, yt_t=yt_t, hp=hp):
                        def ep():
                            for h in (hp, hp + 1):
                                rcp = normp.tile([128, 512], F32, tag="rcp")
                                nc.vector.reciprocal_approx_fast(rcp[:], rs_ps[h][:])
                                nc.vector.tensor_tensor(
                                    yt_t[:, h, :], o_ps[h][:], rcp[:],
                                    AluOpType.mult)
                        return ep
                    pending_epilogue[0] = make_epilogue()

                # ---------------- phase 3 (per t): projection for s-tiles 4t..4t+3
                flush_epilogue()
                for si in range(4 * t, 4 * t + 4):
                    sl = si - 4 * t
                    for dtp in range(2):
                        # pj0 borrows the attention-score psum pool (idle
                        # during proj) so consecutive groups double-buffer
                        pj0 = psS.tile([128, 512], F32, tag="S")
                        pj1 = psB.tile([128, 512], F32, tag="B")
                        for h in range(NH):
                            lhs = yt_t[:, h, sl * 128:(sl + 1) * 128]
                            nc.tensor.matmul(pj0[:], lhs, wp[2 * dtp][:, h, :],
                                             start=(h == 0), stop=(h == NH - 1),
                                             skip_group_check=True)
                            nc.tensor.matmul(pj1[:], lhs, wp[2 * dtp + 1][:, h, :],
                                             start=(h == 0), stop=(h == NH - 1),
                                             skip_group_check=True)
                        ev = outsb.tile([128, 1024], BF16, tag="ev")
                        nc.scalar.copy(ev[:, 0:512], pj0[:])
                        nc.vector.tensor_copy(ev[:, 512:1024], pj1[:])
                        nc.sync.dma_start(
                            Y[si * 128:(si + 1) * 128,
                              dtp * 1024:(dtp + 1) * 1024], ev[:])

    nc.compile()
    return nc


def _host_inputs(x, Wq, Wk, Wv, Wproj, q_gain):
    import ml_dtypes
    bf16 = ml_dtypes.bfloat16

    x = np.asarray(x, dtype=np.float32)
    Wq = np.asarray(Wq, dtype=np.float32)
    Wk = np.asarray(Wk, dtype=np.float32)
    Wv = np.asarray(Wv, dtype=np.float32)
    Wproj = np.asarray(Wproj, dtype=np.float32)
    q_gain = np.asarray(q_gain, dtype=np.float32)

    inv = (1.0 / ROPE_BASE ** (np.arange(0, HD, 2, dtype=np.float32) / HD)).astype(np.float32)
    ang = np.outer(np.arange(S, dtype=np.float32), inv)
    cos = np.cos(ang).astype(np.float32)
    sin = np.sin(ang).astype(np.float32)
    cos2 = np.concatenate([cos, cos], 1).reshape(NT, 128, HD).transpose(1, 0, 2).copy()
    sin2 = np.concatenate([sin, -sin], 1).reshape(NT, 128, HD).transpose(1, 0, 2).copy()

    qq = np.arange(128)[None, :]
    kk = np.arange(128)[:, None]
    tri = (kk <= qq).astype(bf16)
    ident = np.eye(128, dtype=bf16)
    onesq = np.ones((128, 128), dtype=bf16)

    # pre-tile x[b].T as [si4, c4, p, a, s] (contiguous 4KB per partition line)
    xTb = [np.ascontiguousarray(
        x[b].T.astype(bf16).reshape(4, 4, 128, 4, 512).transpose(3, 0, 2, 1, 4))
        for b in range(B)]

    in_maps = []
    for cid in range(8):
        b, g = cid // 4, cid % 4
        wq = Wq[g * 512:(g + 1) * 512, :].T            # [D, 512]
        wk = Wk[g * 128:(g + 1) * 128, :].T            # [D, 128]
        wv = Wv[g * 128:(g + 1) * 128, :].T
        wkv = np.concatenate([wk, wv], 1)              # [D, 256]
        wp = Wproj[:, g * 512:(g + 1) * 512].T         # [512, D] (c_local, dout)
        in_maps.append({
            "XT": xTb[b],
            "WQ": np.ascontiguousarray(
                wq.reshape(NC_, 128, 512).transpose(1, 0, 2)).astype(bf16),
            "WKV": np.ascontiguousarray(
                wkv.reshape(NC_, 128, 256).transpose(1, 0, 2)).astype(bf16),
            "WP": np.ascontiguousarray(
                wp.reshape(NH, 128, 4, 512).transpose(2, 1, 0, 3)).astype(bf16),
            "COS2": cos2, "SIN2": sin2,
            "GSM": (q_gain[g * 4:(g + 1) * 4] * SM).reshape(1, NH).astype(np.float32),
            "IDENT": ident, "ONESQ": onesq, "TRI": tri,
        })
    return in_maps


def _get_prog():
    global _PROG
    if _PROG is None:
        _PROG = _build_program()
    return _PROG


def kernel(x, Wq, Wk, Wv, Wproj, q_gain, _trace=False, _tmpdir=None):
    from concourse.bass_utils import run_bass_kernel_spmd
    nc = _get_prog()
    in_maps = _host_inputs(x, Wq, Wk, Wv, Wproj, q_gain)
    kwargs = {}
    if _tmpdir is not None:
        os.makedirs(_tmpdir, exist_ok=True)
        kwargs["tmpdir"] = _tmpdir
    res = run_bass_kernel_spmd(nc, in_maps, list(range(8)), trace=_trace, **kwargs)
    y = np.empty((B, S, D), dtype=np.float32)
    for b in range(B):
        acc = res.results[4 * b]["Y"].astype(np.float32)
        for g in range(1, 4):
            acc = acc + res.results[4 * b + g]["Y"].astype(np.float32)
        y[b] = acc
    if _trace:
        kernel._last_result = res
    return y


# revision 33
# speedup vs baseline: 1.7266x; 1.0391x over previous
"""Causal GQA self-attention (B=2, S=2048, D=2048, H=16, KV=4) on 8 TRN2 cores.

Sharding: core = (b, g) with b = batch (2) x g = kv-head group (4).
Each core computes 4 q-heads / 1 kv-head for one batch and a partial
projection output [S, D] in bf16; host sums the 4 group partials per batch.

v2 changes vs baseline (496us):
  - all matmul operands bf16 (x, W, qT/kT/v, probs, yt, Wp); psum stays fp32.
  - epilogue: reciprocal on [1,512] BEFORE broadcast (was reciprocal of
    [128,512] = 56us DVE); chain is recip -> gpsimd bcast -> one DVE mult.
  - proj eviction via ACT copy (psum->bf16 sbuf) + dma; no DVE in proj path.
  - rope: batched ops across the 4 q heads via broadcast APs (3 DVE + 1
    gpsimd per s-tile instead of 12 DVE + 4 gpsimd).
  - causal diagonal trim: diag blocks only compute live q columns.
  - software-pipelined: transposes delayed one s-tile; attention exp/pv
    chain pipelined so PE never waits on ACT exp.
  - weight/x DMA split across 4 engine queues; first matmul after ~1.3MB.
"""
import os
import sys

if '/opt/trn_rl_repo' not in sys.path:
    sys.path.insert(0, '/opt/trn_rl_repo')

import numpy as np

B, S, D = 2, 2048, 2048
NH_TOT, NKV_TOT, HD = 16, 4, 128
NH = 4                 # q heads per core
NT = S // 128          # 16 s-tiles
NC_ = D // 128         # 16 c-tiles
T = 4                  # q-slices of 512
SM = 1.0 / np.sqrt(HD)
EPS = float(np.finfo(np.float32).eps)
ROPE_BASE = 10000.0

_PROG = None


def _build_program():
    import concourse.bass as bass
    import concourse.mybir as mybir
    import concourse.tile as tile
    from concourse import bacc
    from concourse.alu_op_type import AluOpType

    F32 = mybir.dt.float32
    BF16 = mybir.dt.bfloat16
    AF = mybir.ActivationFunctionType

    nc = bacc.Bacc("TRN2", target_bir_lowering=False, debug=False)

    # All inputs pre-tiled on host so every dma_start reads a fully
    # CONTIGUOUS dram block (strided sources measured ~3x slower).
    # x[b].T: [si4, c4, p, a, s]; each (si4,c4) load = contiguous 512KB
    XT = nc.dram_tensor("XT", [4, 4, 128, 4, 512], BF16, kind="ExternalInput")
    WQ = nc.dram_tensor("WQ", [4, 128, 4, 512], BF16, kind="ExternalInput")   # [c4, c_p, a, dq]
    WKV = nc.dram_tensor("WKV", [4, 128, 4, 256], BF16, kind="ExternalInput")
    WP = nc.dram_tensor("WP", [4, 128, NH, 512], BF16, kind="ExternalInput")  # [dq, c_in_head, h, dout]
    CS = nc.dram_tensor("CS", [2, 128, NT, HD], BF16, kind="ExternalInput")   # cos;sin
    GSM = nc.dram_tensor("GSM", [1, NH], F32, kind="ExternalInput")           # gain*sm per head
    CONST3 = nc.dram_tensor("CONST3", [128, 3, 128], BF16, kind="ExternalInput")  # ident|onesq|tri
    Y = nc.dram_tensor("Y", [NT, 2, 128, 1024], BF16, kind="ExternalOutput")

    with tile.TileContext(nc) as tc:
        with (
            tc.tile_pool(name="const", bufs=1) as const,
            tc.tile_pool(name="w", bufs=4) as wpool,
            tc.tile_pool(name="stream", bufs=3) as stream,
            tc.tile_pool(name="small", bufs=3) as small,
            tc.tile_pool(name="norm", bufs=4) as normp,
            tc.tile_pool(name="rope", bufs=2) as ropep,
            tc.tile_pool(name="big", bufs=1) as big,
            tc.tile_pool(name="yt", bufs=2) as ytp,
            tc.tile_pool(name="probs", bufs=4) as probsp,
            tc.tile_pool(name="outsb", bufs=4) as outsb,
            tc.tile_pool(name="psA", bufs=2, space="PSUM") as psA,
            tc.tile_pool(name="psB", bufs=2, space="PSUM") as psB,
            tc.tile_pool(name="psS", bufs=2, space="PSUM") as psS,
        ):
            # --- startup DMA. Queues deliver ~0.1-0.13 MB/us each, so the
            # first s-tile is weight/x-DMA paced: deliver in fine pieces,
            # spread across the three DMA-capable queues, and consume c-tiles
            # in arrival order (ci_order below matches queue assignment).
            wq_sb = [wpool.tile([128, 4, 512], BF16, tag="wq", name=f"wq{i}")
                     for i in range(4)]
            wkv_sb = [wpool.tile([128, 4, 256], BF16, tag="wkv", name=f"wkv{i}")
                      for i in range(4)]
            xs4 = []
            xs0 = stream.tile([128, NC_, 512], BF16, tag="xs")
            # sync: xs0 c0/c1, WQ3, WKV3; gpsimd: WQ2, xs0 c2/c3, WKV2, WP;
            # scalar: consts, WQ0, WQ1, cos/sin, WKV0, WKV1
            nc.sync.dma_start(xs0[:, 0:4, :], XT[0, 0])
            nc.sync.dma_start(xs0[:, 4:8, :], XT[0, 1])
            nc.sync.dma_start(wq_sb[3][:], WQ[3])
            nc.sync.dma_start(wkv_sb[3][:], WKV[3])
            xs4.append(xs0)

            gsm = const.tile([1, NH], F32)
            nc.scalar.dma_start(gsm[:], GSM[:])
            const3 = const.tile([128, 3, 128], BF16)
            nc.scalar.dma_start(const3[:], CONST3[:])
            ident = const3[:, 0, :]
            onesq = const3[:, 1, :]
            tri = const3[:, 2, :]
            nc.scalar.dma_start(wq_sb[0][:], WQ[0])
            nc.scalar.dma_start(wq_sb[1][:], WQ[1])
            cs = const.tile([128, 2, NT, HD], BF16)
            nc.scalar.dma_start(cs[:, 0], CS[0])
            nc.scalar.dma_start(cs[:, 1], CS[1])
            cos2 = cs[:, 0]
            sin2 = cs[:, 1]
            nc.scalar.dma_start(wkv_sb[0][:], WKV[0])
            nc.scalar.dma_start(wkv_sb[1][:], WKV[1])

            nc.gpsimd.dma_start(wq_sb[2][:], WQ[2])
            nc.gpsimd.dma_start(xs0[:, 8:12, :], XT[0, 2])
            nc.gpsimd.dma_start(xs0[:, 12:16, :], XT[0, 3])
            nc.gpsimd.dma_start(wkv_sb[2][:], WKV[2])
            gsm_bc = const.tile([128, NH], F32)
            nc.gpsimd.partition_broadcast(gsm_bc[:], gsm[:])
            wp = [wpool.tile([128, NH, 512], BF16, tag="wp", name=f"wp{i}")
                  for i in range(4)]

            ci_order = list(range(NC_))

            qT = big.tile([128, NH, S], BF16)
            kT = big.tile([128, S], BF16)
            v_nat = big.tile([128, NT, HD], BF16)

            # ---------------- phase 1: QKV + rms-norm + rope + transpose
            pending_tp = [None]

            def flush_tp():
                if pending_tp[0] is not None:
                    pending_tp[0]()
                    pending_tp[0] = None

            for si in range(NT):
                c4i = si // 4
                sl = si % 4
                if si == 0:
                    # prefetch: xs1 on sync, xs2 on gpsimd
                    for s4, eng in ((1, nc.sync), (2, nc.gpsimd)):
                        nxt = stream.tile([128, NC_, 512], BF16, tag="xs",
                                          name=f"xs{s4}")
                        for c4 in range(4):
                            eng.dma_start(nxt[:, 4 * c4:4 * c4 + 4, :], XT[s4, c4])
                        xs4.append(nxt)
                elif si == 4:
                    nxt = stream.tile([128, NC_, 512], BF16, tag="xs", name="xs3")
                    for c4 in range(4):
                        nc.gpsimd.dma_start(nxt[:, 4 * c4:4 * c4 + 4, :], XT[3, c4])
                    xs4.append(nxt)
                elif si == 8:
                    for dq in range(4):
                        nc.gpsimd.dma_start(wp[dq][:], WP[dq])
                xs = xs4[c4i]
                q_ps = psA.tile([128, 512], F32, tag="A")
                kv_ps = psB.tile([128, 256], F32, tag="B")
                for i, ci in enumerate(ci_order):
                    nc.tensor.matmul(q_ps[:], xs[:, ci, sl * 128:(sl + 1) * 128],
                                     wq_sb[ci // 4][:, ci % 4, :],
                                     start=(i == 0), stop=(i == NC_ - 1))
                for i, ci in enumerate(ci_order):
                    nc.tensor.matmul(kv_ps[:], xs[:, ci, sl * 128:(sl + 1) * 128],
                                     wkv_sb[ci // 4][:, ci % 4, :],
                                     start=(i == 0), stop=(i == NC_ - 1))

                # sum of squares per head (q: 4 heads, k: 1) on ACT
                scr = small.tile([128, 128], F32, tag="scr")
                ssq = small.tile([128, 8], F32, tag="ssq")
                for h in range(NH):
                    nc.scalar.activation(scr[:], q_ps[:, h * 128:(h + 1) * 128],
                                         AF.Square, accum_out=ssq[:, h:h + 1])
                nc.scalar.activation(scr[:], kv_ps[:, 0:128], AF.Square,
                                     accum_out=ssq[:, 4:5])
                mn = small.tile([128, 8], F32, tag="mn")
                nc.vector.tensor_scalar(mn[:, 0:5], ssq[:, 0:5], 1.0 / HD, EPS,
                                        AluOpType.mult, AluOpType.add)
                rt = small.tile([128, 8], F32, tag="rt")
                nc.scalar.sqrt(rt[:, 0:5], mn[:, 0:5])
                rn = small.tile([128, 8], F32, tag="rn")
                nc.vector.reciprocal(rn[:, 0:5], rt[:, 0:5])
                qsc = small.tile([128, 4], F32, tag="qsc")
                nc.vector.tensor_tensor(qsc[:], rn[:, 0:4], gsm_bc[:], AluOpType.mult)

                # batched rope for the 4 q heads:
                #   qs  = q_ps * qsc[head]      (scale, per-head broadcast AP)
                #   tcs = qs * cos[rep]         (full width)
                #   tsn = swap_halves(qs) * sin[rep]  (two half ops)
                #   nat = tcs + tsn  -> bf16    (gpsimd)
                qs = ropep.tile([128, 4, 128], F32, tag="qs")
                qsc_b = qsc[:, 0:4].unsqueeze(2).broadcast_to([128, 4, 128])
                q3 = q_ps[:].rearrange("p (h d) -> p h d", h=4)
                nc.vector.tensor_tensor(qs[:], q3, qsc_b, AluOpType.mult)
                cos_b = cos2[:, si, :].unsqueeze(1).broadcast_to([128, 4, 128])
                tcs = ropep.tile([128, 4, 128], F32, tag="tcs")
                nc.vector.tensor_tensor(tcs[:], qs[:], cos_b, AluOpType.mult)
                tsn = ropep.tile([128, 4, 128], F32, tag="tsn")
                sinA = sin2[:, si, 0:64].unsqueeze(1).broadcast_to([128, 4, 64])
                sinB = sin2[:, si, 64:128].unsqueeze(1).broadcast_to([128, 4, 64])
                nc.vector.tensor_tensor(tsn[:, :, 0:64], qs[:, :, 64:128], sinA,
                                        AluOpType.mult)
                nc.vector.tensor_tensor(tsn[:, :, 64:128], qs[:, :, 0:64], sinB,
                                        AluOpType.mult)
                natq = ropep.tile([128, 4, 128], BF16, tag="natq")
                nc.gpsimd.tensor_tensor(natq[:], tcs[:], tsn[:], AluOpType.add)

                # k rope (1 head): fused scale via scalar_tensor_tensor
                kcs = ropep.tile([128, 128], F32, tag="kcs")
                ksn = ropep.tile([128, 128], F32, tag="ksn")
                kraw = kv_ps[:, 0:128]
                nc.vector.scalar_tensor_tensor(
                    kcs[:], kraw, rn[:, 4:5], cos2[:, si, :], AluOpType.mult,
                    AluOpType.mult)
                nc.vector.scalar_tensor_tensor(
                    ksn[:, 0:64], kraw[:, 64:128], rn[:, 4:5], sin2[:, si, 0:64],
                    AluOpType.mult, AluOpType.mult)
                nc.vector.scalar_tensor_tensor(
                    ksn[:, 64:128], kraw[:, 0:64], rn[:, 4:5], sin2[:, si, 64:128],
                    AluOpType.mult, AluOpType.mult)
                natk = ropep.tile([128, 128], BF16, tag="natk")
                nc.gpsimd.tensor_tensor(natk[:], kcs[:], ksn[:], AluOpType.add)

                # v: ACT copy out of psum (fp32 -> bf16)
                nc.scalar.copy(v_nat[:, si, :], kv_ps[:, 128:256])

                # transposes delayed one s-tile so PE never waits on rope
                flush_tp()

                def make_tp(si=si, natq=natq, natk=natk):
                    def tp_():
                        for h in range(NH):
                            tp = psS.tile([128, 128], BF16, tag="S", name=f"tpq_{si}_{h}")
                            nc.tensor.transpose(tp[:], natq[:, h, :], ident[:])
                            nc.vector.tensor_copy(qT[:, h, si * 128:(si + 1) * 128], tp[:])
                        tp = psS.tile([128, 128], BF16, tag="S", name=f"tpk_{si}")
                        nc.tensor.transpose(tp[:], natk[:], ident[:])
                        nc.vector.tensor_copy(kT[:, si * 128:(si + 1) * 128], tp[:])
                    return tp_
                pending_tp[0] = make_tp()
            # last s-tile's transposes are flushed after the first attention
            # unit (they are first needed by t=3), hiding the rope tail

            # ---------------- phase 2: attention, two heads interleaved,
            # software-pipelined: emit sc(j), exp(j), then pv/rs(j-1).
            pending_epilogue = [None]

            def flush_epilogue():
                if pending_epilogue[0] is not None:
                    pending_epilogue[0]()
                    pending_epilogue[0] = None

            for t in range(T):
                yt_t = ytp.tile([128, NH, 512], BF16, tag="yt")
                nblk = 4 * t + 4
                for hp in (0, 2):
                    o_ps = {}
                    rs_ps = {}
                    for h in (hp, hp + 1):
                        o_ps[h] = psA.tile([128, 512], F32, tag="A", name=f"o_ps_{t}_{h}")
                        rs_ps[h] = psB.tile([128, 512], F32, tag="B", name=f"rs_ps_{t}_{h}")

                    sc_tiles = {}
                    prb_tiles = {}

                    def emit_sc(j, t=t, hp=hp, sc_tiles=sc_tiles, prb_tiles=prb_tiles):
                        off = j - 4 * t
                        q0 = max(off, 0) * 128   # live q start within the slice
                        sc = psS.tile([128, 1024], F32, tag="S", name=f"sc_{t}_{hp}_{j}")
                        for u, h in enumerate((hp, hp + 1)):
                            nc.tensor.matmul(
                                sc[:, u * 512 + q0:(u + 1) * 512],
                                kT[:, j * 128:(j + 1) * 128],
                                qT[:, h, t * 512 + q0:(t + 1) * 512],
                                start=True, stop=True, skip_group_check=True)
                        prb = probsp.tile([128, 1024], BF16, tag="probs",
                                          name=f"prb_{t}_{hp}_{j}")
                        if off <= 0:
                            nc.scalar.activation(prb[:], sc[:], AF.Exp)
                        else:
                            for u in range(2):
                                nc.scalar.activation(
                                    prb[:, u * 512 + q0:(u + 1) * 512],
                                    sc[:, u * 512 + q0:(u + 1) * 512], AF.Exp)
                        if off >= 0:
                            # mask the 128-wide diagonal sub-block
                            for u in range(2):
                                nc.vector.tensor_tensor(
                                    prb[:, u * 512 + q0:u * 512 + q0 + 128],
                                    prb[:, u * 512 + q0:u * 512 + q0 + 128],
                                    tri[:], AluOpType.mult)
                        sc_tiles[j] = sc
                        prb_tiles[j] = prb

                    def emit_pvrs(j, t=t, hp=hp, o_ps=o_ps, rs_ps=rs_ps,
                                  prb_tiles=prb_tiles, nblk=nblk):
                        off = j - 4 * t
                        q0 = max(off, 0) * 128
                        prb = prb_tiles.pop(j)
                        first = (j == 0)
                        last = (j == nblk - 1)
                        for u, h in enumerate((hp, hp + 1)):
                            nc.tensor.matmul(
                                o_ps[h][:, q0:512], v_nat[:, j, :],
                                prb[:, u * 512 + q0:(u + 1) * 512],
                                start=first, stop=last, skip_group_check=True)
                            # all-ones stationary: row sums land pre-broadcast
                            # on all 128 partitions
                            nc.tensor.matmul(
                                rs_ps[h][:, q0:512], onesq[:],
                                prb[:, u * 512 + q0:(u + 1) * 512],
                                start=first, stop=last, skip_group_check=True)

                    depth = 3
                    for j in range(depth):
                        emit_sc(j)
                    for j in range(depth, nblk):
                        emit_sc(j)
                        emit_pvrs(j - depth)
                        if j == depth:
                            flush_epilogue()
                    for j in range(nblk - depth, nblk):
                        emit_pvrs(j)
                        if nblk == depth and j == nblk - depth:
                            flush_epilogue()
                    if t == 0 and hp == 0:
                        flush_tp()

                    # normalize: rs is already broadcast across partitions by
                    # the all-ones matmul; fast approx reciprocal (18 bits,
                    # plenty for bf16 output) then one multiply
                    def make_epilogue(o_ps=o_ps, rs_ps=rs_ps, yt_t=yt_t, hp=hp):
                        def ep():
                            for h in (hp, hp + 1):
                                rcp = normp.tile([128, 512], F32, tag="rcp")
                                nc.vector.reciprocal_approx_fast(rcp[:], rs_ps[h][:])
                                nc.vector.tensor_tensor(
                                    yt_t[:, h, :], o_ps[h][:], rcp[:],
                                    AluOpType.mult)
                        return ep
                    pending_epilogue[0] = make_epilogue()

                # ---------------- phase 3 (per t): projection for s-tiles 4t..4t+3
                flush_epilogue()
                for si in range(4 * t, 4 * t + 4):
                    sl = si - 4 * t
                    for dtp in range(2):
                        # pj0 borrows the attention-score psum pool (idle
                        # during proj) so consecutive groups double-buffer
                        pj0 = psS.tile([128, 512], F32, tag="S")
                        pj1 = psB.tile([128, 512], F32, tag="B")
                        for h in range(NH):
                            lhs = yt_t[:, h, sl * 128:(sl + 1) * 128]
                            nc.tensor.matmul(pj0[:], lhs, wp[2 * dtp][:, h, :],
                                             start=(h == 0), stop=(h == NH - 1),
                                             skip_group_check=True)
                            nc.tensor.matmul(pj1[:], lhs, wp[2 * dtp + 1][:, h, :],
                                             start=(h == 0), stop=(h == NH - 1),
                                             skip_group_check=True)
                        ev = outsb.tile([128, 1024], BF16, tag="ev")
                        nc.scalar.copy(ev[:, 0:512], pj0[:])
                        nc.vector.tensor_copy(ev[:, 512:1024], pj1[:])
                        nc.sync.dma_start(Y[si, dtp], ev[:])

    nc.compile()
    return nc


def _host_inputs(x, Wq, Wk, Wv, Wproj, q_gain):
    import ml_dtypes
    bf16 = ml_dtypes.bfloat16

    x = np.asarray(x, dtype=np.float32)
    Wq = np.asarray(Wq, dtype=np.float32)
    Wk = np.asarray(Wk, dtype=np.float32)
    Wv = np.asarray(Wv, dtype=np.float32)
    Wproj = np.asarray(Wproj, dtype=np.float32)
    q_gain = np.asarray(q_gain, dtype=np.float32)

    inv = (1.0 / ROPE_BASE ** (np.arange(0, HD, 2, dtype=np.float32) / HD)).astype(np.float32)
    ang = np.outer(np.arange(S, dtype=np.float32), inv)
    cos = np.cos(ang).astype(np.float32)
    sin = np.sin(ang).astype(np.float32)
    cos2 = np.concatenate([cos, cos], 1).reshape(NT, 128, HD).transpose(1, 0, 2)
    sin2 = np.concatenate([sin, -sin], 1).reshape(NT, 128, HD).transpose(1, 0, 2)
    cs = np.ascontiguousarray(np.stack([cos2, sin2])).astype(bf16)  # [2,128,NT,HD]

    qq = np.arange(128)[None, :]
    kk = np.arange(128)[:, None]
    const3 = np.ascontiguousarray(np.stack(
        [np.eye(128, dtype=np.float32),
         np.ones((128, 128), dtype=np.float32),
         (kk <= qq).astype(np.float32)], axis=1)).astype(bf16)  # [128,3,128]

    # pre-tile x[b].T as [si4, c4, p, a, s] (contiguous 512KB per load)
    xTb = [np.ascontiguousarray(
        x[b].T.astype(bf16).reshape(4, 4, 128, 4, 512).transpose(3, 0, 2, 1, 4))
        for b in range(B)]

    in_maps = []
    for cid in range(8):
        b, g = cid // 4, cid % 4
        wq = Wq[g * 512:(g + 1) * 512, :].T            # [D, 512]
        wk = Wk[g * 128:(g + 1) * 128, :].T            # [D, 128]
        wv = Wv[g * 128:(g + 1) * 128, :].T
        wkv = np.concatenate([wk, wv], 1)              # [D, 256]
        wp = Wproj[:, g * 512:(g + 1) * 512].T         # [512, D] (c_local, dout)
        in_maps.append({
            "XT": xTb[b],
            # [c4, p, a, dq]: each [1] slice is a contiguous 512KB block
            "WQ": np.ascontiguousarray(
                wq.reshape(4, 4, 128, 512).transpose(0, 2, 1, 3)).astype(bf16),
            "WKV": np.ascontiguousarray(
                wkv.reshape(4, 4, 128, 256).transpose(0, 2, 1, 3)).astype(bf16),
            "WP": np.ascontiguousarray(
                wp.reshape(NH, 128, 4, 512).transpose(2, 1, 0, 3)).astype(bf16),
            "CS": cs,
            "GSM": (q_gain[g * 4:(g + 1) * 4] * SM).reshape(1, NH).astype(np.float32),
            "CONST3": const3,
        })
    return in_maps


def _get_prog():
    global _PROG
    if _PROG is None:
        _PROG = _build_program()
    return _PROG


def kernel(x, Wq, Wk, Wv, Wproj, q_gain, _trace=False, _tmpdir=None):
    from concourse.bass_utils import run_bass_kernel_spmd
    nc = _get_prog()
    in_maps = _host_inputs(x, Wq, Wk, Wv, Wproj, q_gain)
    kwargs = {}
    if _tmpdir is not None:
        os.makedirs(_tmpdir, exist_ok=True)
        kwargs["tmpdir"] = _tmpdir
    res = run_bass_kernel_spmd(nc, in_maps, list(range(8)), trace=_trace, **kwargs)
    y = np.empty((B, S, D), dtype=np.float32)
    for b in range(B):
        acc = res.results[4 * b]["Y"].astype(np.float32)
        for g in range(1, 4):
            acc = acc + res.results[4 * b + g]["Y"].astype(np.float32)
        # Y is [si, dtp, p, c] -> [S, D]
        y[b] = acc.transpose(0, 2, 1, 3).reshape(S, D)
    if _trace:
        kernel._last_result = res
    return y
